# revision 17
# baseline (speedup 1.0000x reference)
"""Complex batch-norm Trainium2 kernel (nn_ComplexBatchNormal).

Full inputs: x_real/x_imag [16, 32, 256, 256] f32, params [32, 256, 256] f32.
Output: complex64 [16, 32, 256, 256].

Sharding: channels C=32 split across 8 cores (4 channels each) -> fully local
batch statistics per core, no collectives.

Per-core algorithm (positions N = 4*256*256 = 262144, batch B = 16):
  pass 1: S_r, S_i, S_rr, S_ii, S_ri per position, accumulated over B via
          TensorE identity-matmuls into PSUM (ScalarE squares, VectorE product).
  coef:   analytic inverse-sqrt of the 2x2 covariance; fold gamma/beta/mu into
          6 per-position coefficients a1,a2,a0,b1,b2,b0 with
          out_r = a1*x_r + a2*x_i + a0, out_i = b1*x_r + b2*x_i + b0.
  pass 2: bf16, batched over half the batch per instruction with step-0
          broadcast APs for the coefficients; the final adds emit f32
          interleaved (re,im) pairs so the DRAM output is directly complex64.
"""

import sys

if "/opt/trn_rl_repo" not in sys.path:
    sys.path.insert(0, "/opt/trn_rl_repo")

from contextlib import ExitStack

import numpy as np

import concourse.bacc as bacc
import concourse.bass as bass
import concourse.tile as tile
from concourse import masks, mybir
from concourse.bass_utils import run_bass_kernel_spmd

P = 128          # SBUF partitions
F = 512          # free-dim positions per tile (= one PSUM bank of f32)
NB = 16          # batch size
HB = NB // 2     # half-batch group for batched pass-2 ops
EPS = 1e-5
N_CORES = 8
C_FULL = 32
C_LOC = C_FULL // N_CORES  # 4 channels per core
HW = 256 * 256
NPOS_FULL = C_LOC * HW     # 262144 positions per core

f32 = mybir.dt.float32
bf16 = mybir.dt.float16  # 16-bit compute dtype for pass 2 (fp16: 10-bit mantissa)


def bcast_free(ap: bass.AP, n: int) -> bass.AP:
    """View [P, F] as [P, n, F] with the middle dim broadcast (step 0)."""
    return bass.AP(tensor=ap.tensor, offset=ap.offset, ap=[ap.ap[0], [0, n], ap.ap[1]])


def _emit(
    nc: bacc.Bacc,
    ctx: ExitStack,
    tc: "tile.TileContext",
    npos: int,
    finals_engine: str = "gpsimd",
):
    NT = npos // (P * F)
    assert NT * P * F == npos

    xr_d = nc.dram_tensor("xr", [NB, npos], f32, kind="ExternalInput")
    xi_d = nc.dram_tensor("xi", [NB, npos], f32, kind="ExternalInput")
    grr_d = nc.dram_tensor("grr", [npos], f32, kind="ExternalInput")
    gri_d = nc.dram_tensor("gri", [npos], f32, kind="ExternalInput")
    gii_d = nc.dram_tensor("gii", [npos], f32, kind="ExternalInput")
    bet_d = nc.dram_tensor("bet", [npos], f32, kind="ExternalInput")
    out_d = nc.dram_tensor("out", [NB, 2 * npos], f32, kind="ExternalOutput")

    G = 4  # batch-samples per load group
    xr_gv = xr_d.ap().rearrange("(g q) (t p f) -> g t p q f", q=G, p=P, f=F)
    xi_gv = xi_d.ap().rearrange("(g q) (t p f) -> g t p q f", q=G, p=P, f=F)
    grr_v = grr_d.ap().rearrange("(t p f) -> t p f", p=P, f=F)
    gri_v = gri_d.ap().rearrange("(t p f) -> t p f", p=P, f=F)
    gii_v = gii_d.ap().rearrange("(t p f) -> t p f", p=P, f=F)
    bet_v = bet_d.ap().rearrange("(t p f) -> t p f", p=P, f=F)
    out_v = out_d.ap().rearrange("b (t p f) -> b t p f", p=P, f=2 * F)

    singles = ctx.enter_context(tc.tile_pool(name="singles", bufs=1))
    xpool = ctx.enter_context(tc.tile_pool(name="x", bufs=2))     # f32 staging groups
    xbpool = ctx.enter_context(tc.tile_pool(name="xb", bufs=2))   # fp16 resident
    sqpool = ctx.enter_context(tc.tile_pool(name="sq", bufs=1))
    gpool = ctx.enter_context(tc.tile_pool(name="g", bufs=1))
    cpool = ctx.enter_context(tc.tile_pool(name="coef", bufs=1))
    cbpool = ctx.enter_context(tc.tile_pool(name="coefb", bufs=1))
    wpool = ctx.enter_context(tc.tile_pool(name="w", bufs=2))
    opool = ctx.enter_context(tc.tile_pool(name="o", bufs=2))
    psum = ctx.enter_context(tc.tile_pool(name="ps", bufs=1, space="PSUM"))

    ident = singles.tile([P, P], f32)
    masks.make_identity(nc, ident[:])
    identb = singles.tile([P, P], bf16)
    nc.scalar.copy(identb[:], ident[:])

    inv16 = 1.0 / NB
    ACT = mybir.ActivationFunctionType

    for t in range(NT):
        # --- params for this position tile ---
        grr = gpool.tile([P, F], f32, tag="grr", name=f"grr{t}")
        gri = gpool.tile([P, F], f32, tag="gri", name=f"gri{t}")
        gii = gpool.tile([P, F], f32, tag="gii", name=f"gii{t}")
        bet = gpool.tile([P, F], f32, tag="bet", name=f"bet{t}")
        nc.sync.dma_start(grr[:], grr_v[t])
        nc.sync.dma_start(gri[:], gri_v[t])
        nc.sync.dma_start(gii[:], gii_v[t])
        nc.sync.dma_start(bet[:], bet_v[t])

        # --- pass 1: load x, cast to bf16, accumulate 5 stats over B in PSUM ---
        S_r = psum.tile([P, F], f32, tag="S_r", name=f"S_r{t}")
        S_i = psum.tile([P, F], f32, tag="S_i", name=f"S_i{t}")
        S_rr = psum.tile([P, F], f32, tag="S_rr", name=f"S_rr{t}")
        S_ii = psum.tile([P, F], f32, tag="S_ii", name=f"S_ii{t}")
        S_ri = psum.tile([P, F], f32, tag="S_ri", name=f"S_ri{t}")

        XB = xbpool.tile([P, NB, F], bf16, tag="XB", name=f"XB{t}")
        XIB = xbpool.tile([P, NB, F], bf16, tag="XIB", name=f"XIB{t}")

        for g in range(NB // G):
            xg = xpool.tile([P, G, F], f32, tag="xr", name=f"xr{t}_{g}")
            nc.sync.dma_start(xg[:], xr_gv[g, t])
            yg = xpool.tile([P, G, F], f32, tag="xi", name=f"xi{t}_{g}")
            nc.sync.dma_start(yg[:], xi_gv[g, t])

            XBg = XB[:, g * G : (g + 1) * G, :]
            XIBg = XIB[:, g * G : (g + 1) * G, :]
            nc.scalar.copy(XBg, xg[:])
            nc.scalar.copy(XIBg, yg[:])

            sq_r = sqpool.tile([P, G, F], bf16, tag="sqr", name=f"sqr{t}_{g}")
            sq_i = sqpool.tile([P, G, F], bf16, tag="sqi", name=f"sqi{t}_{g}")
            if t == 0:
                # DVE is idle during the first tile's stats: square there
                nc.vector.tensor_mul(sq_r[:], XBg, XBg)
                nc.vector.tensor_mul(sq_i[:], XIBg, XIBg)
            else:
                nc.scalar.square(sq_r[:], xg[:])
                nc.scalar.square(sq_i[:], yg[:])
            p_g = sqpool.tile([P, G, F], bf16, tag="pg", name=f"pg{t}_{g}")
            nc.vector.tensor_mul(p_g[:], XBg, XIBg)

            for q in range(G):
                b = g * G + q
                st = b == 0
                sp = b == NB - 1
                nc.tensor.matmul(S_r[:], identb[:], XB[:, b, :], start=st, stop=sp)
                nc.tensor.matmul(S_i[:], identb[:], XIB[:, b, :], start=st, stop=sp)
                nc.tensor.matmul(S_rr[:], identb[:], sq_r[:, q, :], start=st, stop=sp)
                nc.tensor.matmul(S_ii[:], identb[:], sq_i[:, q, :], start=st, stop=sp)
                nc.tensor.matmul(S_ri[:], identb[:], p_g[:, q, :], start=st, stop=sp)

        # --- coefficient phase (per-position math on [P, F] f32 tiles) ---
        cp = lambda tag: cpool.tile([P, F], f32, tag=tag, name=f"{tag}{t}")
        scr = lambda i: cpool.tile([P, F], f32, tag=f"scr{i}", name=f"scr{i}_{t}")

        mu_r = cp("mu_r")
        nc.scalar.activation(mu_r[:], S_r[:], ACT.Copy, scale=inv16)
        mu_i = cp("mu_i")
        nc.scalar.activation(mu_i[:], S_i[:], ACT.Copy, scale=inv16)
        Vrr = cp("Vrr")
        nc.scalar.activation(Vrr[:], S_rr[:], ACT.Copy, bias=EPS, scale=inv16)
        Vii = cp("Vii")
        nc.scalar.activation(Vii[:], S_ii[:], ACT.Copy, bias=EPS, scale=inv16)
        Vri = cp("Vri")
        nc.scalar.activation(Vri[:], S_ri[:], ACT.Copy, scale=inv16)

        mr2 = scr(0)
        nc.scalar.square(mr2[:], mu_r[:])
        nc.vector.tensor_sub(Vrr[:], Vrr[:], mr2[:])
        mi2 = scr(1)
        nc.scalar.square(mi2[:], mu_i[:])
        nc.vector.tensor_sub(Vii[:], Vii[:], mi2[:])
        mri = scr(2)
        nc.vector.tensor_mul(mri[:], mu_r[:], mu_i[:])
        nc.vector.tensor_sub(Vri[:], Vri[:], mri[:])

        tau = scr(3)
        nc.vector.tensor_add(tau[:], Vrr[:], Vii[:])
        det = scr(4)
        nc.vector.tensor_mul(det[:], Vrr[:], Vii[:])
        vri2 = scr(5)
        nc.scalar.square(vri2[:], Vri[:])
        nc.vector.tensor_sub(det[:], det[:], vri2[:])

        s_s = cp("s_s")
        nc.scalar.sqrt(s_s[:], det[:])
        # tau <- tau + 2*s in one fused op
        nc.vector.scalar_tensor_tensor(
            tau[:], s_s[:], 2.0, tau[:], mybir.AluOpType.mult, mybir.AluOpType.add
        )
        t_t = cp("t_t")
        nc.scalar.sqrt(t_t[:], tau[:])

        st_ = scr(0)
        nc.vector.tensor_mul(st_[:], s_s[:], t_t[:])
        inv = cp("inv")
        nc.vector.reciprocal(inv[:], st_[:])

        # W matrix (Wri holds +Vri*inv; true Wri = -that)
        Wrr = cp("Wrr")
        nc.vector.tensor_add(Vii[:], Vii[:], s_s[:])
        nc.vector.tensor_mul(Wrr[:], Vii[:], inv[:])
        Wii = cp("Wii")
        nc.vector.tensor_add(Vrr[:], Vrr[:], s_s[:])
        nc.vector.tensor_mul(Wii[:], Vrr[:], inv[:])
        Wri = cp("Wri")
        nc.vector.tensor_mul(Wri[:], Vri[:], inv[:])

        # output coefficients
        a1 = cp("a1")
        nc.vector.tensor_mul(a1[:], grr[:], Wrr[:])
        m2 = cp("m2")
        nc.vector.tensor_mul(m2[:], gri[:], Wri[:])
        nc.vector.tensor_sub(a1[:], a1[:], m2[:])

        a1b = cbpool.tile([P, F], bf16, tag="a1b", name=f"a1b{t}")
        nc.scalar.copy(a1b[:], a1[:])

        a2 = cp("a2")
        nc.vector.tensor_mul(a2[:], gri[:], Wii[:])
        m4 = scr(1)
        nc.vector.tensor_mul(m4[:], grr[:], Wri[:])
        nc.vector.tensor_sub(a2[:], a2[:], m4[:])

        a2b = cbpool.tile([P, F], bf16, tag="a2b", name=f"a2b{t}")
        nc.scalar.copy(a2b[:], a2[:])

        b1 = cp("b1")
        nc.vector.tensor_mul(b1[:], gri[:], Wrr[:])
        m6 = scr(2)
        nc.vector.tensor_mul(m6[:], gii[:], Wri[:])
        nc.vector.tensor_sub(b1[:], b1[:], m6[:])

        b1b = cbpool.tile([P, F], bf16, tag="b1b", name=f"b1b{t}")
        nc.scalar.copy(b1b[:], b1[:])

        b2 = cp("b2")
        nc.vector.tensor_mul(b2[:], gii[:], Wii[:])
        nc.vector.tensor_sub(b2[:], b2[:], m2[:])

        b2b = cbpool.tile([P, F], bf16, tag="b2b", name=f"b2b{t}")
        nc.scalar.copy(b2b[:], b2[:])

        a0 = cpool.tile([P, F], f32, tag="a0", name=f"a0{t}", bufs=2)
        n1 = scr(3)
        nc.vector.tensor_mul(n1[:], a1[:], mu_r[:])
        nc.vector.tensor_sub(a0[:], bet[:], n1[:])
        n2 = scr(4)
        nc.vector.tensor_mul(n2[:], a2[:], mu_i[:])
        nc.vector.tensor_sub(a0[:], a0[:], n2[:])

        a0b = cbpool.tile([P, F], bf16, tag="a0b", name=f"a0b{t}")
        nc.scalar.copy(a0b[:], a0[:])

        b0 = cpool.tile([P, F], f32, tag="b0", name=f"b0{t}", bufs=2)
        n3 = scr(5)
        nc.vector.tensor_mul(n3[:], b1[:], mu_r[:])
        nc.vector.tensor_sub(b0[:], bet[:], n3[:])
        n4 = scr(0)
        nc.vector.tensor_mul(n4[:], b2[:], mu_i[:])
        nc.vector.tensor_sub(b0[:], b0[:], n4[:])


        b0b = cbpool.tile([P, F], bf16, tag="b0b", name=f"b0b{t}")
        nc.scalar.copy(b0b[:], b0[:])

        # --- pass 2: batched bf16, half the batch per instruction ---
        for h in range(2):
            b0_ = h * HB
            XBh = XB[:, b0_ : b0_ + HB, :]
            XIBh = XIB[:, b0_ : b0_ + HB, :]

            U = wpool.tile([P, HB, F], bf16, tag="U", name=f"U{t}_{h}")
            nc.vector.tensor_mul(U[:], XBh, bcast_free(a1b[:], HB))
            V = wpool.tile([P, HB, F], bf16, tag="V", name=f"V{t}_{h}", bufs=1)
            nc.vector.tensor_mul(V[:], XIBh, bcast_free(a2b[:], HB))
            nc.vector.tensor_add(U[:], U[:], V[:])
            nc.vector.tensor_add(U[:], U[:], bcast_free(a0b[:], HB))

            U2 = wpool.tile([P, HB, F], bf16, tag="U", name=f"U2{t}_{h}")
            nc.vector.tensor_mul(U2[:], XBh, bcast_free(b1b[:], HB))
            V2 = wpool.tile([P, HB, F], bf16, tag="V", name=f"V2{t}_{h}", bufs=1)
            nc.vector.tensor_mul(V2[:], XIBh, bcast_free(b2b[:], HB))
            nc.vector.tensor_add(U2[:], U2[:], V2[:])
            nc.vector.tensor_add(U2[:], U2[:], bcast_free(b0b[:], HB))

            nfin = 0
            for bb in range(HB):
                b = b0_ + bb
                out_c = opool.tile([P, 2 * F], f32, tag="oc", name=f"oc{t}_{b}")
                oc = out_c.rearrange("p (f two) -> p f two", two=2)
                dve_share = 3 if t < NT - 1 else 6
                for comp, Uc in enumerate((U, U2)):
                    # strided fp16->f32 interleave copy; split ACT/DVE
                    if nfin % 8 < 8 - dve_share:
                        nc.scalar.copy(oc[:, :, comp], Uc[:, bb, :])
                    else:
                        nc.vector.tensor_copy(oc[:, :, comp], Uc[:, bb, :])
                    nfin += 1
                nc.sync.dma_start(out_v[b, t], out_c[:])


def build_nc(npos: int = NPOS_FULL, finals_engine: str = "gpsimd") -> bacc.Bacc:
    nc = bacc.Bacc("TRN2", target_bir_lowering=False, debug=False)
    with tile.TileContext(nc) as tc:
        with ExitStack() as ctx:
            _emit(nc, ctx, tc, npos, finals_engine=finals_engine)
    nc.compile()
    return nc


_cache: dict = {}


def _get_nc(npos: int, finals_engine: str = "gpsimd") -> bacc.Bacc:
    key = (npos, finals_engine)
    if key not in _cache:
        _cache[key] = build_nc(npos, finals_engine)
    return _cache[key]


def make_in_maps(x_real, x_imag, gamma_rr, gamma_ri, gamma_ii, beta):
    """Shard channels across cores; returns per-core input dicts."""
    in_maps = []
    for k in range(N_CORES):
        sl = slice(k * C_LOC, (k + 1) * C_LOC)
        in_maps.append(
            {
                "xr": np.ascontiguousarray(x_real[:, sl]).reshape(NB, -1),
                "xi": np.ascontiguousarray(x_imag[:, sl]).reshape(NB, -1),
                "grr": np.ascontiguousarray(gamma_rr[sl]).reshape(-1),
                "gri": np.ascontiguousarray(gamma_ri[sl]).reshape(-1),
                "gii": np.ascontiguousarray(gamma_ii[sl]).reshape(-1),
                "bet": np.ascontiguousarray(beta[sl]).reshape(-1),
            }
        )
    return in_maps


def assemble_output(results) -> np.ndarray:
    """Gather per-core interleaved f32 outputs into the full complex64 array."""
    B = NB
    out = np.empty((B, C_FULL, HW), dtype=np.complex64)
    for k in range(N_CORES):
        o = np.asarray(results[k]["out"])  # [B, 2*NPOS] f32
        oc = o.view(np.complex64).reshape(B, C_LOC, HW)
        out[:, k * C_LOC : (k + 1) * C_LOC] = oc
    return out.reshape(B, C_FULL, 256, 256)


def kernel(x_real, x_imag, gamma_rr, gamma_ri, gamma_ii, beta) -> np.ndarray:
    x_real = np.asarray(x_real, dtype=np.float32)
    x_imag = np.asarray(x_imag, dtype=np.float32)
    gamma_rr = np.asarray(gamma_rr, dtype=np.float32)
    gamma_ri = np.asarray(gamma_ri, dtype=np.float32)
    gamma_ii = np.asarray(gamma_ii, dtype=np.float32)
    beta = np.asarray(beta, dtype=np.float32)

    nc = _get_nc(NPOS_FULL)
    in_maps = make_in_maps(x_real, x_imag, gamma_rr, gamma_ri, gamma_ii, beta)
    res = run_bass_kernel_spmd(nc, in_maps, core_ids=list(range(N_CORES)))
    return assemble_output(res.results)


# revision 18
# speedup vs baseline: 1.0049x; 1.0049x over previous
"""Complex batch-norm Trainium2 kernel (nn_ComplexBatchNormal).

Full inputs: x_real/x_imag [16, 32, 256, 256] f32, params [32, 256, 256] f32.
Output: complex64 [16, 32, 256, 256].

Sharding: channels C=32 split across 8 cores (4 channels each) -> fully local
batch statistics per core, no collectives.

Per-core algorithm (positions N = 4*256*256 = 262144, batch B = 16):
  pass 1: S_r, S_i, S_rr, S_ii, S_ri per position, accumulated over B via
          TensorE identity-matmuls into PSUM (ScalarE squares, VectorE product).
  coef:   analytic inverse-sqrt of the 2x2 covariance; fold gamma/beta/mu into
          6 per-position coefficients a1,a2,a0,b1,b2,b0 with
          out_r = a1*x_r + a2*x_i + a0, out_i = b1*x_r + b2*x_i + b0.
  pass 2: fp16, batched over half the batch per instruction with step-0
          broadcast APs for the coefficients (DVE 2x mode); bias folded into
          the batched chain; strided fp16->f32 cast-copies (split ScalarE/
          VectorE) emit interleaved (re,im) pairs so the DRAM output is
          directly complex64.
"""

import sys

if "/opt/trn_rl_repo" not in sys.path:
    sys.path.insert(0, "/opt/trn_rl_repo")

from contextlib import ExitStack

import numpy as np

import concourse.bacc as bacc
import concourse.bass as bass
import concourse.tile as tile
from concourse import masks, mybir
from concourse.bass_utils import run_bass_kernel_spmd

P = 128          # SBUF partitions
F = 512          # free-dim positions per tile (= one PSUM bank of f32)
NB = 16          # batch size
HB = NB // 2     # half-batch group for batched pass-2 ops
EPS = 1e-5
N_CORES = 8
C_FULL = 32
C_LOC = C_FULL // N_CORES  # 4 channels per core
HW = 256 * 256
NPOS_FULL = C_LOC * HW     # 262144 positions per core

f32 = mybir.dt.float32
bf16 = mybir.dt.float16  # 16-bit compute dtype for pass 2 (fp16: 10-bit mantissa)


def bcast_free(ap: bass.AP, n: int) -> bass.AP:
    """View [P, F] as [P, n, F] with the middle dim broadcast (step 0)."""
    return bass.AP(tensor=ap.tensor, offset=ap.offset, ap=[ap.ap[0], [0, n], ap.ap[1]])


def _emit(
    nc: bacc.Bacc,
    ctx: ExitStack,
    tc: "tile.TileContext",
    npos: int,
    finals_engine: str = "gpsimd",
):
    NT = npos // (P * F)
    assert NT * P * F == npos

    xr_d = nc.dram_tensor("xr", [NB, npos], f32, kind="ExternalInput")
    xi_d = nc.dram_tensor("xi", [NB, npos], f32, kind="ExternalInput")
    grr_d = nc.dram_tensor("grr", [npos], f32, kind="ExternalInput")
    gri_d = nc.dram_tensor("gri", [npos], f32, kind="ExternalInput")
    gii_d = nc.dram_tensor("gii", [npos], f32, kind="ExternalInput")
    bet_d = nc.dram_tensor("bet", [npos], f32, kind="ExternalInput")
    out_d = nc.dram_tensor("out", [NB, 2 * npos], f32, kind="ExternalOutput")

    G = 4  # batch-samples per load group
    xr_gv = xr_d.ap().rearrange("(g q) (t p f) -> g t p q f", q=G, p=P, f=F)
    xi_gv = xi_d.ap().rearrange("(g q) (t p f) -> g t p q f", q=G, p=P, f=F)
    grr_v = grr_d.ap().rearrange("(t p f) -> t p f", p=P, f=F)
    gri_v = gri_d.ap().rearrange("(t p f) -> t p f", p=P, f=F)
    gii_v = gii_d.ap().rearrange("(t p f) -> t p f", p=P, f=F)
    bet_v = bet_d.ap().rearrange("(t p f) -> t p f", p=P, f=F)
    out_v = out_d.ap().rearrange("b (t p f) -> b t p f", p=P, f=2 * F)

    singles = ctx.enter_context(tc.tile_pool(name="singles", bufs=1))
    xpool = ctx.enter_context(tc.tile_pool(name="x", bufs=2))     # f32 staging groups
    xbpool = ctx.enter_context(tc.tile_pool(name="xb", bufs=2))   # fp16 resident
    sqpool = ctx.enter_context(tc.tile_pool(name="sq", bufs=1))
    gpool = ctx.enter_context(tc.tile_pool(name="g", bufs=1))
    cpool = ctx.enter_context(tc.tile_pool(name="coef", bufs=1))
    cbpool = ctx.enter_context(tc.tile_pool(name="coefb", bufs=1))
    wpool = ctx.enter_context(tc.tile_pool(name="w", bufs=2))
    opool = ctx.enter_context(tc.tile_pool(name="o", bufs=2))
    psum = ctx.enter_context(tc.tile_pool(name="ps", bufs=1, space="PSUM"))

    ident = singles.tile([P, P], f32)
    masks.make_identity(nc, ident[:])
    identb = singles.tile([P, P], bf16)
    nc.scalar.copy(identb[:], ident[:])

    inv16 = 1.0 / NB
    ACT = mybir.ActivationFunctionType

    for t in range(NT):
        # --- params for this position tile ---
        grr = gpool.tile([P, F], f32, tag="grr", name=f"grr{t}")
        gri = gpool.tile([P, F], f32, tag="gri", name=f"gri{t}")
        gii = gpool.tile([P, F], f32, tag="gii", name=f"gii{t}")
        bet = gpool.tile([P, F], f32, tag="bet", name=f"bet{t}")
        nc.sync.dma_start(grr[:], grr_v[t])
        nc.sync.dma_start(gri[:], gri_v[t])
        nc.sync.dma_start(gii[:], gii_v[t])
        nc.sync.dma_start(bet[:], bet_v[t])

        # --- pass 1: load x, cast to bf16, accumulate 5 stats over B in PSUM ---
        S_r = psum.tile([P, F], f32, tag="S_r", name=f"S_r{t}")
        S_i = psum.tile([P, F], f32, tag="S_i", name=f"S_i{t}")
        S_rr = psum.tile([P, F], f32, tag="S_rr", name=f"S_rr{t}")
        S_ii = psum.tile([P, F], f32, tag="S_ii", name=f"S_ii{t}")
        S_ri = psum.tile([P, F], f32, tag="S_ri", name=f"S_ri{t}")

        XB = xbpool.tile([P, NB, F], bf16, tag="XB", name=f"XB{t}")
        XIB = xbpool.tile([P, NB, F], bf16, tag="XIB", name=f"XIB{t}")

        for g in range(NB // G):
            xg = xpool.tile([P, G, F], f32, tag="xr", name=f"xr{t}_{g}")
            nc.sync.dma_start(xg[:], xr_gv[g, t])
            yg = xpool.tile([P, G, F], f32, tag="xi", name=f"xi{t}_{g}")
            nc.sync.dma_start(yg[:], xi_gv[g, t])

            XBg = XB[:, g * G : (g + 1) * G, :]
            XIBg = XIB[:, g * G : (g + 1) * G, :]
            nc.scalar.copy(XBg, xg[:])
            nc.scalar.copy(XIBg, yg[:])

            sq_r = sqpool.tile([P, G, F], bf16, tag="sqr", name=f"sqr{t}_{g}")
            sq_i = sqpool.tile([P, G, F], bf16, tag="sqi", name=f"sqi{t}_{g}")
            if t == 0:
                # DVE is idle during the first tile's stats: square there
                nc.vector.tensor_mul(sq_r[:], XBg, XBg)
                nc.vector.tensor_mul(sq_i[:], XIBg, XIBg)
            else:
                nc.scalar.square(sq_r[:], xg[:])
                nc.scalar.square(sq_i[:], yg[:])
            p_g = sqpool.tile([P, G, F], bf16, tag="pg", name=f"pg{t}_{g}")
            nc.vector.tensor_mul(p_g[:], XBg, XIBg)

            for q in range(G):
                b = g * G + q
                st = b == 0
                sp = b == NB - 1
                nc.tensor.matmul(S_r[:], identb[:], XB[:, b, :], start=st, stop=sp)
                nc.tensor.matmul(S_i[:], identb[:], XIB[:, b, :], start=st, stop=sp)
                nc.tensor.matmul(S_rr[:], identb[:], sq_r[:, q, :], start=st, stop=sp)
                nc.tensor.matmul(S_ii[:], identb[:], sq_i[:, q, :], start=st, stop=sp)
                nc.tensor.matmul(S_ri[:], identb[:], p_g[:, q, :], start=st, stop=sp)

        # --- coefficient phase (per-position math on [P, F] f32 tiles) ---
        cp = lambda tag: cpool.tile([P, F], f32, tag=tag, name=f"{tag}{t}")
        scr = lambda i: cpool.tile([P, F], f32, tag=f"scr{i}", name=f"scr{i}_{t}")

        mu_r = cp("mu_r")
        nc.scalar.activation(mu_r[:], S_r[:], ACT.Copy, scale=inv16)
        mu_i = cp("mu_i")
        nc.scalar.activation(mu_i[:], S_i[:], ACT.Copy, scale=inv16)
        Vrr = cp("Vrr")
        nc.scalar.activation(Vrr[:], S_rr[:], ACT.Copy, bias=EPS, scale=inv16)
        Vii = cp("Vii")
        nc.scalar.activation(Vii[:], S_ii[:], ACT.Copy, bias=EPS, scale=inv16)
        Vri = cp("Vri")
        nc.scalar.activation(Vri[:], S_ri[:], ACT.Copy, scale=inv16)

        mr2 = scr(0)
        nc.scalar.square(mr2[:], mu_r[:])
        nc.vector.tensor_sub(Vrr[:], Vrr[:], mr2[:])
        mi2 = scr(1)
        nc.scalar.square(mi2[:], mu_i[:])
        nc.vector.tensor_sub(Vii[:], Vii[:], mi2[:])
        mri = scr(2)
        nc.vector.tensor_mul(mri[:], mu_r[:], mu_i[:])
        nc.vector.tensor_sub(Vri[:], Vri[:], mri[:])

        tau = scr(3)
        nc.vector.tensor_add(tau[:], Vrr[:], Vii[:])
        det = scr(4)
        nc.vector.tensor_mul(det[:], Vrr[:], Vii[:])
        vri2 = scr(5)
        nc.scalar.square(vri2[:], Vri[:])
        nc.vector.tensor_sub(det[:], det[:], vri2[:])

        s_s = cp("s_s")
        nc.scalar.sqrt(s_s[:], det[:])
        # tau <- tau + 2*s in one fused op
        nc.vector.scalar_tensor_tensor(
            tau[:], s_s[:], 2.0, tau[:], mybir.AluOpType.mult, mybir.AluOpType.add
        )
        t_t = cp("t_t")
        nc.scalar.sqrt(t_t[:], tau[:])

        st_ = scr(0)
        nc.vector.tensor_mul(st_[:], s_s[:], t_t[:])
        inv = cp("inv")
        nc.vector.reciprocal(inv[:], st_[:])

        # W matrix (Wri holds +Vri*inv; true Wri = -that)
        Wrr = cp("Wrr")
        nc.vector.tensor_add(Vii[:], Vii[:], s_s[:])
        nc.vector.tensor_mul(Wrr[:], Vii[:], inv[:])
        Wii = cp("Wii")
        nc.vector.tensor_add(Vrr[:], Vrr[:], s_s[:])
        nc.vector.tensor_mul(Wii[:], Vrr[:], inv[:])
        Wri = cp("Wri")
        nc.vector.tensor_mul(Wri[:], Vri[:], inv[:])

        # output coefficients
        a1 = cp("a1")
        nc.vector.tensor_mul(a1[:], grr[:], Wrr[:])
        m2 = cp("m2")
        nc.vector.tensor_mul(m2[:], gri[:], Wri[:])
        nc.vector.tensor_sub(a1[:], a1[:], m2[:])

        a1b = cbpool.tile([P, F], bf16, tag="a1b", name=f"a1b{t}")
        nc.scalar.copy(a1b[:], a1[:])

        a2 = cp("a2")
        nc.vector.tensor_mul(a2[:], gri[:], Wii[:])
        m4 = scr(1)
        nc.vector.tensor_mul(m4[:], grr[:], Wri[:])
        nc.vector.tensor_sub(a2[:], a2[:], m4[:])

        a2b = cbpool.tile([P, F], bf16, tag="a2b", name=f"a2b{t}")
        nc.scalar.copy(a2b[:], a2[:])

        b1 = cp("b1")
        nc.vector.tensor_mul(b1[:], gri[:], Wrr[:])
        m6 = scr(2)
        nc.vector.tensor_mul(m6[:], gii[:], Wri[:])
        nc.vector.tensor_sub(b1[:], b1[:], m6[:])

        b1b = cbpool.tile([P, F], bf16, tag="b1b", name=f"b1b{t}")
        nc.scalar.copy(b1b[:], b1[:])

        b2 = cp("b2")
        nc.vector.tensor_mul(b2[:], gii[:], Wii[:])
        nc.vector.tensor_sub(b2[:], b2[:], m2[:])

        b2b = cbpool.tile([P, F], bf16, tag="b2b", name=f"b2b{t}")
        nc.scalar.copy(b2b[:], b2[:])

        a0 = cpool.tile([P, F], f32, tag="a0", name=f"a0{t}", bufs=2)
        n1 = scr(3)
        nc.vector.tensor_mul(n1[:], a1[:], mu_r[:])
        nc.vector.tensor_sub(a0[:], bet[:], n1[:])
        n2 = scr(4)
        nc.vector.tensor_mul(n2[:], a2[:], mu_i[:])
        nc.vector.tensor_sub(a0[:], a0[:], n2[:])

        a0b = cbpool.tile([P, F], bf16, tag="a0b", name=f"a0b{t}")
        nc.scalar.copy(a0b[:], a0[:])

        b0 = cpool.tile([P, F], f32, tag="b0", name=f"b0{t}", bufs=2)
        n3 = scr(5)
        nc.vector.tensor_mul(n3[:], b1[:], mu_r[:])
        nc.vector.tensor_sub(b0[:], bet[:], n3[:])
        n4 = scr(0)
        nc.vector.tensor_mul(n4[:], b2[:], mu_i[:])
        nc.vector.tensor_sub(b0[:], b0[:], n4[:])


        b0b = cbpool.tile([P, F], bf16, tag="b0b", name=f"b0b{t}")
        nc.scalar.copy(b0b[:], b0[:])

        # --- pass 2: batched bf16, half the batch per instruction ---
        for h in range(2):
            b0_ = h * HB
            XBh = XB[:, b0_ : b0_ + HB, :]
            XIBh = XIB[:, b0_ : b0_ + HB, :]

            U = wpool.tile([P, HB, F], bf16, tag="U", name=f"U{t}_{h}")
            nc.vector.tensor_mul(U[:], XBh, bcast_free(a1b[:], HB))
            V = wpool.tile([P, HB, F], bf16, tag="V", name=f"V{t}_{h}", bufs=1)
            nc.vector.tensor_mul(V[:], XIBh, bcast_free(a2b[:], HB))
            nc.vector.tensor_add(U[:], U[:], V[:])
            nc.vector.tensor_add(U[:], U[:], bcast_free(a0b[:], HB))

            U2 = wpool.tile([P, HB, F], bf16, tag="U", name=f"U2{t}_{h}")
            nc.vector.tensor_mul(U2[:], XBh, bcast_free(b1b[:], HB))
            V2 = wpool.tile([P, HB, F], bf16, tag="V", name=f"V2{t}_{h}", bufs=1)
            nc.vector.tensor_mul(V2[:], XIBh, bcast_free(b2b[:], HB))
            nc.vector.tensor_add(U2[:], U2[:], V2[:])
            nc.vector.tensor_add(U2[:], U2[:], bcast_free(b0b[:], HB))

            nfin = 0
            for bb in range(HB):
                b = b0_ + bb
                out_c = opool.tile([P, 2 * F], f32, tag="oc", name=f"oc{t}_{b}")
                oc = out_c.rearrange("p (f two) -> p f two", two=2)
                dve_share = 4 if t < NT - 1 else 7
                for comp, Uc in enumerate((U, U2)):
                    # strided fp16->f32 interleave copy; split ACT/DVE
                    if nfin % 8 < 8 - dve_share:
                        nc.scalar.copy(oc[:, :, comp], Uc[:, bb, :])
                    else:
                        nc.vector.tensor_copy(oc[:, :, comp], Uc[:, bb, :])
                    nfin += 1
                nc.sync.dma_start(out_v[b, t], out_c[:])


def build_nc(npos: int = NPOS_FULL, finals_engine: str = "gpsimd") -> bacc.Bacc:
    nc = bacc.Bacc("TRN2", target_bir_lowering=False, debug=False)
    with tile.TileContext(nc) as tc:
        with ExitStack() as ctx:
            _emit(nc, ctx, tc, npos, finals_engine=finals_engine)
    nc.compile()
    return nc


_cache: dict = {}


def _get_nc(npos: int, finals_engine: str = "gpsimd") -> bacc.Bacc:
    key = (npos, finals_engine)
    if key not in _cache:
        _cache[key] = build_nc(npos, finals_engine)
    return _cache[key]


def make_in_maps(x_real, x_imag, gamma_rr, gamma_ri, gamma_ii, beta):
    """Shard channels across cores; returns per-core input dicts."""
    in_maps = []
    for k in range(N_CORES):
        sl = slice(k * C_LOC, (k + 1) * C_LOC)
        in_maps.append(
            {
                "xr": np.ascontiguousarray(x_real[:, sl]).reshape(NB, -1),
                "xi": np.ascontiguousarray(x_imag[:, sl]).reshape(NB, -1),
                "grr": np.ascontiguousarray(gamma_rr[sl]).reshape(-1),
                "gri": np.ascontiguousarray(gamma_ri[sl]).reshape(-1),
                "gii": np.ascontiguousarray(gamma_ii[sl]).reshape(-1),
                "bet": np.ascontiguousarray(beta[sl]).reshape(-1),
            }
        )
    return in_maps


def assemble_output(results) -> np.ndarray:
    """Gather per-core interleaved f32 outputs into the full complex64 array."""
    B = NB
    out = np.empty((B, C_FULL, HW), dtype=np.complex64)
    for k in range(N_CORES):
        o = np.asarray(results[k]["out"])  # [B, 2*NPOS] f32
        oc = o.view(np.complex64).reshape(B, C_LOC, HW)
        out[:, k * C_LOC : (k + 1) * C_LOC] = oc
    return out.reshape(B, C_FULL, 256, 256)


def kernel(x_real, x_imag, gamma_rr, gamma_ri, gamma_ii, beta) -> np.ndarray:
    x_real = np.asarray(x_real, dtype=np.float32)
    x_imag = np.asarray(x_imag, dtype=np.float32)
    gamma_rr = np.asarray(gamma_rr, dtype=np.float32)
    gamma_ri = np.asarray(gamma_ri, dtype=np.float32)
    gamma_ii = np.asarray(gamma_ii, dtype=np.float32)
    beta = np.asarray(beta, dtype=np.float32)

    nc = _get_nc(NPOS_FULL)
    in_maps = make_in_maps(x_real, x_imag, gamma_rr, gamma_ri, gamma_ii, beta)
    res = run_bass_kernel_spmd(nc, in_maps, core_ids=list(range(N_CORES)))
    return assemble_output(res.results)


# revision 21
# speedup vs baseline: 1.0373x; 1.0322x over previous
"""Complex batch-norm Trainium2 kernel (nn_ComplexBatchNormal).

Full inputs: x_real/x_imag [16, 32, 256, 256] f32, params [32, 256, 256] f32.
Output: complex64 [16, 32, 256, 256].

Sharding: channels C=32 split across 8 cores (4 channels each) -> fully local
batch statistics per core, no collectives.

Per-core algorithm (positions N = 4*256*256 = 262144, batch B = 16):
  pass 1: S_r, S_i, S_rr, S_ii, S_ri per position, accumulated over B via
          TensorE identity-matmuls into PSUM (ScalarE squares, VectorE product).
  coef:   analytic inverse-sqrt of the 2x2 covariance; fold gamma/beta/mu into
          6 per-position coefficients a1,a2,a0,b1,b2,b0 with
          out_r = a1*x_r + a2*x_i + a0, out_i = b1*x_r + b2*x_i + b0.
  pass 2: fp16, batched over half the batch per instruction with step-0
          broadcast APs for the coefficients (DVE 2x mode); bias folded into
          the batched chain; strided fp16->f32 cast-copies (split ScalarE/
          VectorE) emit interleaved (re,im) pairs so the DRAM output is
          directly complex64.
"""

import sys

if "/opt/trn_rl_repo" not in sys.path:
    sys.path.insert(0, "/opt/trn_rl_repo")

from contextlib import ExitStack

import numpy as np

import concourse.bacc as bacc
import concourse.bass as bass
import concourse.tile as tile
from concourse import masks, mybir
from concourse.bass_utils import run_bass_kernel_spmd

P = 128          # SBUF partitions
F = 512          # free-dim positions per tile (= one PSUM bank of f32)
NB = 16          # batch size
HB = NB // 2     # half-batch group for batched pass-2 ops
EPS = 1e-5
N_CORES = 8
C_FULL = 32
C_LOC = C_FULL // N_CORES  # 4 channels per core
HW = 256 * 256
NPOS_FULL = C_LOC * HW     # 262144 positions per core

f32 = mybir.dt.float32
bf16 = mybir.dt.float16  # 16-bit compute dtype for pass 2 (fp16: 10-bit mantissa)


def bcast_free(ap: bass.AP, n: int) -> bass.AP:
    """View [P, F] as [P, n, F] with the middle dim broadcast (step 0)."""
    return bass.AP(tensor=ap.tensor, offset=ap.offset, ap=[ap.ap[0], [0, n], ap.ap[1]])


def _emit(
    nc: bacc.Bacc,
    ctx: ExitStack,
    tc: "tile.TileContext",
    npos: int,
    finals_engine: str = "gpsimd",
):
    NT = npos // (P * F)
    assert NT * P * F == npos

    xr_d = nc.dram_tensor("xr", [NB, npos], f32, kind="ExternalInput")
    xi_d = nc.dram_tensor("xi", [NB, npos], f32, kind="ExternalInput")
    grr_d = nc.dram_tensor("grr", [npos], f32, kind="ExternalInput")
    gri_d = nc.dram_tensor("gri", [npos], f32, kind="ExternalInput")
    gii_d = nc.dram_tensor("gii", [npos], f32, kind="ExternalInput")
    bet_d = nc.dram_tensor("bet", [npos], f32, kind="ExternalInput")
    out_d = nc.dram_tensor("out", [NB, 2 * npos], f32, kind="ExternalOutput")

    G = 4  # batch-samples per load group
    xr_gv = xr_d.ap().rearrange("(g q) (t p f) -> g t p q f", q=G, p=P, f=F)
    xi_gv = xi_d.ap().rearrange("(g q) (t p f) -> g t p q f", q=G, p=P, f=F)
    grr_v = grr_d.ap().rearrange("(t p f) -> t p f", p=P, f=F)
    gri_v = gri_d.ap().rearrange("(t p f) -> t p f", p=P, f=F)
    gii_v = gii_d.ap().rearrange("(t p f) -> t p f", p=P, f=F)
    bet_v = bet_d.ap().rearrange("(t p f) -> t p f", p=P, f=F)
    out_v = out_d.ap().rearrange("b (t p f) -> b t p f", p=P, f=2 * F)

    singles = ctx.enter_context(tc.tile_pool(name="singles", bufs=1))
    xpool = ctx.enter_context(tc.tile_pool(name="x", bufs=2))     # f32 staging groups
    xbpool = ctx.enter_context(tc.tile_pool(name="xb", bufs=2))   # fp16 resident
    sqpool = ctx.enter_context(tc.tile_pool(name="sq", bufs=1))
    gpool = ctx.enter_context(tc.tile_pool(name="g", bufs=1))
    cpool = ctx.enter_context(tc.tile_pool(name="coef", bufs=1))
    cbpool = ctx.enter_context(tc.tile_pool(name="coefb", bufs=1))
    wpool = ctx.enter_context(tc.tile_pool(name="w", bufs=2))
    opool = ctx.enter_context(tc.tile_pool(name="o", bufs=2))
    psum = ctx.enter_context(tc.tile_pool(name="ps", bufs=1, space="PSUM"))

    ident = singles.tile([P, P], f32)
    masks.make_identity(nc, ident[:])
    identb = singles.tile([P, P], bf16)
    nc.scalar.copy(identb[:], ident[:])

    inv16 = 1.0 / NB
    ACT = mybir.ActivationFunctionType

    for t in range(NT):
        # --- params for this position tile ---
        grr = gpool.tile([P, F], f32, tag="grr", name=f"grr{t}")
        gri = gpool.tile([P, F], f32, tag="gri", name=f"gri{t}")
        gii = gpool.tile([P, F], f32, tag="gii", name=f"gii{t}")
        bet = gpool.tile([P, F], f32, tag="bet", name=f"bet{t}")
        nc.sync.dma_start(grr[:], grr_v[t])
        nc.sync.dma_start(gri[:], gri_v[t])
        nc.sync.dma_start(gii[:], gii_v[t])
        nc.sync.dma_start(bet[:], bet_v[t])

        # --- pass 1: load x, cast to bf16, accumulate 5 stats over B in PSUM ---
        S_r = psum.tile([P, F], f32, tag="S_r", name=f"S_r{t}")
        S_i = psum.tile([P, F], f32, tag="S_i", name=f"S_i{t}")
        S_rr = psum.tile([P, F], f32, tag="S_rr", name=f"S_rr{t}")
        S_ii = psum.tile([P, F], f32, tag="S_ii", name=f"S_ii{t}")
        S_ri = psum.tile([P, F], f32, tag="S_ri", name=f"S_ri{t}")

        XB = xbpool.tile([P, NB, F], bf16, tag="XB", name=f"XB{t}")
        XIB = xbpool.tile([P, NB, F], bf16, tag="XIB", name=f"XIB{t}")

        for g in range(NB // G):
            xg = xpool.tile([P, G, F], f32, tag="xr", name=f"xr{t}_{g}")
            nc.sync.dma_start(xg[:], xr_gv[g, t])
            yg = xpool.tile([P, G, F], f32, tag="xi", name=f"xi{t}_{g}")
            nc.sync.dma_start(yg[:], xi_gv[g, t])

            XBg = XB[:, g * G : (g + 1) * G, :]
            XIBg = XIB[:, g * G : (g + 1) * G, :]
            nc.scalar.copy(XBg, xg[:])
            nc.scalar.copy(XIBg, yg[:])

            sq_r = sqpool.tile([P, G, F], bf16, tag="sqr", name=f"sqr{t}_{g}")
            sq_i = sqpool.tile([P, G, F], bf16, tag="sqi", name=f"sqi{t}_{g}")
            if t == 0:
                # DVE is idle during the first tile's stats: square there
                nc.vector.tensor_mul(sq_r[:], XBg, XBg)
                nc.vector.tensor_mul(sq_i[:], XIBg, XIBg)
            else:
                nc.scalar.square(sq_r[:], xg[:])
                nc.scalar.square(sq_i[:], yg[:])
            p_g = sqpool.tile([P, G, F], bf16, tag="pg", name=f"pg{t}_{g}")
            nc.vector.tensor_mul(p_g[:], XBg, XIBg)

            for q in range(G):
                b = g * G + q
                st = b == 0
                sp = b == NB - 1
                nc.tensor.matmul(S_r[:], identb[:], XB[:, b, :], start=st, stop=sp)
                nc.tensor.matmul(S_i[:], identb[:], XIB[:, b, :], start=st, stop=sp)
                nc.tensor.matmul(S_rr[:], identb[:], sq_r[:, q, :], start=st, stop=sp)
                nc.tensor.matmul(S_ii[:], identb[:], sq_i[:, q, :], start=st, stop=sp)
                nc.tensor.matmul(S_ri[:], identb[:], p_g[:, q, :], start=st, stop=sp)

        # --- coefficient phase (per-position math on [P, F] f32 tiles) ---
        cp = lambda tag: cpool.tile([P, F], f32, tag=tag, name=f"{tag}{t}")
        scr = lambda i: cpool.tile([P, F], f32, tag=f"scr{i}", name=f"scr{i}_{t}")

        mu_r = cp("mu_r")
        nc.scalar.activation(mu_r[:], S_r[:], ACT.Copy, scale=inv16)
        mu_i = cp("mu_i")
        nc.scalar.activation(mu_i[:], S_i[:], ACT.Copy, scale=inv16)
        Vrr = cp("Vrr")
        nc.scalar.activation(Vrr[:], S_rr[:], ACT.Copy, bias=EPS, scale=inv16)
        Vii = cp("Vii")
        nc.scalar.activation(Vii[:], S_ii[:], ACT.Copy, bias=EPS, scale=inv16)
        Vri = cp("Vri")
        nc.scalar.activation(Vri[:], S_ri[:], ACT.Copy, scale=inv16)

        mr2 = scr(0)
        nc.scalar.square(mr2[:], mu_r[:])
        nc.vector.tensor_sub(Vrr[:], Vrr[:], mr2[:])
        mi2 = scr(1)
        nc.scalar.square(mi2[:], mu_i[:])
        nc.vector.tensor_sub(Vii[:], Vii[:], mi2[:])
        mri = scr(2)
        nc.vector.tensor_mul(mri[:], mu_r[:], mu_i[:])
        nc.vector.tensor_sub(Vri[:], Vri[:], mri[:])

        tau = scr(3)
        nc.vector.tensor_add(tau[:], Vrr[:], Vii[:])
        det = scr(4)
        nc.vector.tensor_mul(det[:], Vrr[:], Vii[:])
        vri2 = scr(1)
        nc.scalar.square(vri2[:], Vri[:])
        nc.vector.tensor_sub(det[:], det[:], vri2[:])

        s_s = cp("s_s")
        nc.scalar.sqrt(s_s[:], det[:])
        # tau <- tau + 2*s in one fused op
        nc.vector.scalar_tensor_tensor(
            tau[:], s_s[:], 2.0, tau[:], mybir.AluOpType.mult, mybir.AluOpType.add
        )
        t_t = cp("t_t")
        nc.scalar.sqrt(t_t[:], tau[:])

        st_ = scr(0)
        nc.vector.tensor_mul(st_[:], s_s[:], t_t[:])
        inv = cp("inv")
        nc.vector.reciprocal_approx_fast(inv[:], st_[:])

        # W matrix in place: Wrr <- Vii, Wii <- Vrr, Wri <- Vri
        # (Wri holds +Vri*inv; true Wri = -that)
        nc.vector.tensor_add(Vii[:], Vii[:], s_s[:])
        nc.vector.tensor_mul(Vii[:], Vii[:], inv[:])
        Wrr = Vii
        nc.vector.tensor_add(Vrr[:], Vrr[:], s_s[:])
        nc.vector.tensor_mul(Vrr[:], Vrr[:], inv[:])
        Wii = Vrr
        nc.vector.tensor_mul(Vri[:], Vri[:], inv[:])
        Wri = Vri

        # output coefficients
        a1 = cp("a1")
        nc.vector.tensor_mul(a1[:], grr[:], Wrr[:])
        m2 = cp("m2")
        nc.vector.tensor_mul(m2[:], gri[:], Wri[:])
        nc.vector.tensor_sub(a1[:], a1[:], m2[:])

        a1b = cbpool.tile([P, F], bf16, tag="a1b", name=f"a1b{t}")
        nc.scalar.copy(a1b[:], a1[:])

        a2 = cp("a2")
        nc.vector.tensor_mul(a2[:], gri[:], Wii[:])
        m4 = scr(1)
        nc.vector.tensor_mul(m4[:], grr[:], Wri[:])
        nc.vector.tensor_sub(a2[:], a2[:], m4[:])

        a2b = cbpool.tile([P, F], bf16, tag="a2b", name=f"a2b{t}")
        nc.scalar.copy(a2b[:], a2[:])

        b1 = cp("b1")
        nc.vector.tensor_mul(b1[:], gri[:], Wrr[:])
        m6 = scr(2)
        nc.vector.tensor_mul(m6[:], gii[:], Wri[:])
        nc.vector.tensor_sub(b1[:], b1[:], m6[:])

        b1b = cbpool.tile([P, F], bf16, tag="b1b", name=f"b1b{t}")
        nc.scalar.copy(b1b[:], b1[:])

        b2 = cp("b2")
        nc.vector.tensor_mul(b2[:], gii[:], Wii[:])
        nc.vector.tensor_sub(b2[:], b2[:], m2[:])

        b2b = cbpool.tile([P, F], bf16, tag="b2b", name=f"b2b{t}")
        nc.scalar.copy(b2b[:], b2[:])

        a0 = cpool.tile([P, F], f32, tag="a0", name=f"a0{t}", bufs=2)
        n1 = scr(3)
        nc.vector.tensor_mul(n1[:], a1[:], mu_r[:])
        nc.vector.tensor_sub(a0[:], bet[:], n1[:])
        n2 = scr(4)
        nc.vector.tensor_mul(n2[:], a2[:], mu_i[:])
        nc.vector.tensor_sub(a0[:], a0[:], n2[:])

        a0b = cbpool.tile([P, F], bf16, tag="a0b", name=f"a0b{t}")
        nc.scalar.copy(a0b[:], a0[:])

        b0 = cpool.tile([P, F], f32, tag="b0", name=f"b0{t}", bufs=2)
        n3 = scr(1)
        nc.vector.tensor_mul(n3[:], b1[:], mu_r[:])
        nc.vector.tensor_sub(b0[:], bet[:], n3[:])
        n4 = scr(0)
        nc.vector.tensor_mul(n4[:], b2[:], mu_i[:])
        nc.vector.tensor_sub(b0[:], b0[:], n4[:])


        b0b = cbpool.tile([P, F], bf16, tag="b0b", name=f"b0b{t}")
        nc.scalar.copy(b0b[:], b0[:])

        # --- pass 2: batched bf16, half the batch per instruction ---
        for h in range(2):
            b0_ = h * HB
            XBh = XB[:, b0_ : b0_ + HB, :]
            XIBh = XIB[:, b0_ : b0_ + HB, :]

            U = wpool.tile([P, HB, F], bf16, tag="U", name=f"U{t}_{h}")
            nc.vector.tensor_mul(U[:], XBh, bcast_free(a1b[:], HB))
            nc.vector.tensor_add(U[:], U[:], bcast_free(a0b[:], HB))
            V = wpool.tile([P, HB, F], bf16, tag="V", name=f"V{t}_{h}", bufs=2)
            nc.vector.tensor_mul(V[:], XIBh, bcast_free(a2b[:], HB))

            U2 = wpool.tile([P, HB, F], bf16, tag="U", name=f"U2{t}_{h}")
            nc.vector.tensor_mul(U2[:], XBh, bcast_free(b1b[:], HB))
            nc.vector.tensor_add(U2[:], U2[:], bcast_free(b0b[:], HB))
            V2 = wpool.tile([P, HB, F], bf16, tag="V", name=f"V2{t}_{h}", bufs=2)
            nc.vector.tensor_mul(V2[:], XIBh, bcast_free(b2b[:], HB))

            nfin = 0
            for bb in range(HB):
                b = b0_ + bb
                out_c = opool.tile([P, 2 * F], f32, tag="oc", name=f"oc{t}_{b}")
                oc = out_c.rearrange("p (f two) -> p f two", two=2)
                dve_share = 4 if t < NT - 1 else 7
                for comp, (Uc, Vc) in enumerate(((U, V), (U2, V2))):
                    # U+V summed on the TensorEngine into PSUM
                    PS = psum.tile(
                        [P, F], f32, tag="PS", name=f"PS{t}_{b}_{comp}", bufs=3
                    )
                    nc.tensor.matmul(
                        PS[:], identb[:], Uc[:, bb, :], start=True, stop=False
                    )
                    nc.tensor.matmul(
                        PS[:], identb[:], Vc[:, bb, :], start=False, stop=True
                    )
                    # strided f32 interleave copy from PSUM; split ACT/DVE
                    if nfin % 8 < 8 - dve_share:
                        nc.scalar.copy(oc[:, :, comp], PS[:])
                    else:
                        nc.vector.tensor_copy(oc[:, :, comp], PS[:])
                    nfin += 1
                nc.sync.dma_start(out_v[b, t], out_c[:])


def build_nc(npos: int = NPOS_FULL, finals_engine: str = "gpsimd") -> bacc.Bacc:
    nc = bacc.Bacc("TRN2", target_bir_lowering=False, debug=False)
    with tile.TileContext(nc) as tc:
        with ExitStack() as ctx:
            _emit(nc, ctx, tc, npos, finals_engine=finals_engine)
    nc.compile()
    return nc


_cache: dict = {}


def _get_nc(npos: int, finals_engine: str = "gpsimd") -> bacc.Bacc:
    key = (npos, finals_engine)
    if key not in _cache:
        _cache[key] = build_nc(npos, finals_engine)
    return _cache[key]


def make_in_maps(x_real, x_imag, gamma_rr, gamma_ri, gamma_ii, beta):
    """Shard channels across cores; returns per-core input dicts."""
    in_maps = []
    for k in range(N_CORES):
        sl = slice(k * C_LOC, (k + 1) * C_LOC)
        in_maps.append(
            {
                "xr": np.ascontiguousarray(x_real[:, sl]).reshape(NB, -1),
                "xi": np.ascontiguousarray(x_imag[:, sl]).reshape(NB, -1),
                "grr": np.ascontiguousarray(gamma_rr[sl]).reshape(-1),
                "gri": np.ascontiguousarray(gamma_ri[sl]).reshape(-1),
                "gii": np.ascontiguousarray(gamma_ii[sl]).reshape(-1),
                "bet": np.ascontiguousarray(beta[sl]).reshape(-1),
            }
        )
    return in_maps


def assemble_output(results) -> np.ndarray:
    """Gather per-core interleaved f32 outputs into the full complex64 array."""
    B = NB
    out = np.empty((B, C_FULL, HW), dtype=np.complex64)
    for k in range(N_CORES):
        o = np.asarray(results[k]["out"])  # [B, 2*NPOS] f32
        oc = o.view(np.complex64).reshape(B, C_LOC, HW)
        out[:, k * C_LOC : (k + 1) * C_LOC] = oc
    return out.reshape(B, C_FULL, 256, 256)


def kernel(x_real, x_imag, gamma_rr, gamma_ri, gamma_ii, beta) -> np.ndarray:
    x_real = np.asarray(x_real, dtype=np.float32)
    x_imag = np.asarray(x_imag, dtype=np.float32)
    gamma_rr = np.asarray(gamma_rr, dtype=np.float32)
    gamma_ri = np.asarray(gamma_ri, dtype=np.float32)
    gamma_ii = np.asarray(gamma_ii, dtype=np.float32)
    beta = np.asarray(beta, dtype=np.float32)

    nc = _get_nc(NPOS_FULL)
    in_maps = make_in_maps(x_real, x_imag, gamma_rr, gamma_ri, gamma_ii, beta)
    res = run_bass_kernel_spmd(nc, in_maps, core_ids=list(range(N_CORES)))
    return assemble_output(res.results)


# revision 22
# speedup vs baseline: 1.1156x; 1.0756x over previous
"""Complex batch-norm Trainium2 kernel (nn_ComplexBatchNormal).

Full inputs: x_real/x_imag [16, 32, 256, 256] f32, params [32, 256, 256] f32.
Output: complex64 [16, 32, 256, 256].

Sharding: channels C=32 split across 8 cores (4 channels each) -> fully local
batch statistics per core, no collectives.

Per-core algorithm (positions N = 4*256*256 = 262144, batch B = 16):
  pass 1: S_r, S_i, S_rr, S_ii, S_ri per position, accumulated over B via
          TensorE identity-matmuls into PSUM (ScalarE squares, VectorE product).
  coef:   analytic inverse-sqrt of the 2x2 covariance; fold gamma/beta/mu into
          6 per-position coefficients a1,a2,a0,b1,b2,b0 with
          out_r = a1*x_r + a2*x_i + a0, out_i = b1*x_r + b2*x_i + b0.
  pass 2: fp16, batched over half the batch per instruction with step-0
          broadcast APs for the coefficients (DVE 2x mode); bias folded into
          the batched chain; strided fp16->f32 cast-copies (split ScalarE/
          VectorE) emit interleaved (re,im) pairs so the DRAM output is
          directly complex64.
"""

import sys

if "/opt/trn_rl_repo" not in sys.path:
    sys.path.insert(0, "/opt/trn_rl_repo")

from contextlib import ExitStack

import numpy as np

import concourse.bacc as bacc
import concourse.bass as bass
import concourse.tile as tile
from concourse import masks, mybir
from concourse.bass_utils import run_bass_kernel_spmd

P = 128          # SBUF partitions
F = 512          # free-dim positions per tile (= one PSUM bank of f32)
NB = 16          # batch size
HB = NB // 2     # half-batch group for batched pass-2 ops
EPS = 1e-5
N_CORES = 8
C_FULL = 32
C_LOC = C_FULL // N_CORES  # 4 channels per core
HW = 256 * 256
NPOS_FULL = C_LOC * HW     # 262144 positions per core

f32 = mybir.dt.float32
bf16 = mybir.dt.float16  # 16-bit compute dtype for pass 2 (fp16: 10-bit mantissa)


def bcast_free(ap: bass.AP, n: int) -> bass.AP:
    """View [P, F] as [P, n, F] with the middle dim broadcast (step 0)."""
    return bass.AP(tensor=ap.tensor, offset=ap.offset, ap=[ap.ap[0], [0, n], ap.ap[1]])


def _emit(
    nc: bacc.Bacc,
    ctx: ExitStack,
    tc: "tile.TileContext",
    npos: int,
    finals_engine: str = "gpsimd",
):
    NT = npos // (P * F)
    assert NT * P * F == npos

    xr_d = nc.dram_tensor("xr", [NB, npos], f32, kind="ExternalInput")
    xi_d = nc.dram_tensor("xi", [NB, npos], f32, kind="ExternalInput")
    grr_d = nc.dram_tensor("grr", [npos], f32, kind="ExternalInput")
    gri_d = nc.dram_tensor("gri", [npos], f32, kind="ExternalInput")
    gii_d = nc.dram_tensor("gii", [npos], f32, kind="ExternalInput")
    bet_d = nc.dram_tensor("bet", [npos], f32, kind="ExternalInput")
    out_d = nc.dram_tensor("out", [NB, 2 * npos], f32, kind="ExternalOutput")

    G = 4  # batch-samples per load group
    xr_gv = xr_d.ap().rearrange("(g q) (t p f) -> g t p q f", q=G, p=P, f=F)
    xi_gv = xi_d.ap().rearrange("(g q) (t p f) -> g t p q f", q=G, p=P, f=F)
    grr_v = grr_d.ap().rearrange("(t p f) -> t p f", p=P, f=F)
    gri_v = gri_d.ap().rearrange("(t p f) -> t p f", p=P, f=F)
    gii_v = gii_d.ap().rearrange("(t p f) -> t p f", p=P, f=F)
    bet_v = bet_d.ap().rearrange("(t p f) -> t p f", p=P, f=F)
    out_v = out_d.ap().rearrange("b (t p f) -> b t p f", p=P, f=2 * F)

    singles = ctx.enter_context(tc.tile_pool(name="singles", bufs=1))
    xpool = ctx.enter_context(tc.tile_pool(name="x", bufs=2))     # f32 staging groups
    xbpool = ctx.enter_context(tc.tile_pool(name="xb", bufs=2))   # fp16 resident
    sqpool = ctx.enter_context(tc.tile_pool(name="sq", bufs=1))
    gpool = ctx.enter_context(tc.tile_pool(name="g", bufs=1))
    cpool = ctx.enter_context(tc.tile_pool(name="coef", bufs=1))
    cbpool = ctx.enter_context(tc.tile_pool(name="coefb", bufs=1))
    wpool = ctx.enter_context(tc.tile_pool(name="w", bufs=2))
    opool = ctx.enter_context(tc.tile_pool(name="o", bufs=2))
    psum = ctx.enter_context(tc.tile_pool(name="ps", bufs=1, space="PSUM"))

    ident = singles.tile([P, P], f32)
    masks.make_identity(nc, ident[:])
    identb = singles.tile([P, P], bf16)
    nc.scalar.copy(identb[:], ident[:])

    inv16 = 1.0 / NB
    ACT = mybir.ActivationFunctionType

    for t in range(NT):
        # --- params for this position tile ---
        grr = gpool.tile([P, F], f32, tag="grr", name=f"grr{t}")
        gri = gpool.tile([P, F], f32, tag="gri", name=f"gri{t}")
        gii = gpool.tile([P, F], f32, tag="gii", name=f"gii{t}")
        bet = gpool.tile([P, F], f32, tag="bet", name=f"bet{t}")
        nc.sync.dma_start(grr[:], grr_v[t])
        nc.sync.dma_start(gri[:], gri_v[t])
        nc.sync.dma_start(gii[:], gii_v[t])
        nc.sync.dma_start(bet[:], bet_v[t])

        # --- pass 1: load x, cast to bf16, accumulate 5 stats over B in PSUM ---
        S_r = psum.tile([P, F], f32, tag="S_r", name=f"S_r{t}")
        S_i = psum.tile([P, F], f32, tag="S_i", name=f"S_i{t}")
        S_rr = psum.tile([P, F], f32, tag="S_rr", name=f"S_rr{t}")
        S_ii = psum.tile([P, F], f32, tag="S_ii", name=f"S_ii{t}")
        S_ri = psum.tile([P, F], f32, tag="S_ri", name=f"S_ri{t}")

        XB = xbpool.tile([P, NB, F], bf16, tag="XB", name=f"XB{t}")
        XIB = xbpool.tile([P, NB, F], bf16, tag="XIB", name=f"XIB{t}")

        for g in range(NB // G):
            xg = xpool.tile([P, G, F], f32, tag="xr", name=f"xr{t}_{g}")
            nc.sync.dma_start(xg[:], xr_gv[g, t])
            yg = xpool.tile([P, G, F], f32, tag="xi", name=f"xi{t}_{g}")
            nc.sync.dma_start(yg[:], xi_gv[g, t])

            XBg = XB[:, g * G : (g + 1) * G, :]
            XIBg = XIB[:, g * G : (g + 1) * G, :]
            nc.scalar.copy(XBg, xg[:])
            nc.scalar.copy(XIBg, yg[:])

            sq_r = sqpool.tile([P, G, F], bf16, tag="sqr", name=f"sqr{t}_{g}")
            sq_i = sqpool.tile([P, G, F], bf16, tag="sqi", name=f"sqi{t}_{g}")
            if t == 0:
                # DVE is idle during the first tile's stats: square there
                nc.vector.tensor_mul(sq_r[:], XBg, XBg)
                nc.vector.tensor_mul(sq_i[:], XIBg, XIBg)
            else:
                nc.scalar.square(sq_r[:], xg[:])
                nc.scalar.square(sq_i[:], yg[:])
            p_g = sqpool.tile([P, G, F], bf16, tag="pg", name=f"pg{t}_{g}")
            nc.vector.tensor_mul(p_g[:], XBg, XIBg)

            for q in range(G):
                b = g * G + q
                st = b == 0
                sp = b == NB - 1
                nc.tensor.matmul(S_r[:], identb[:], XB[:, b, :], start=st, stop=sp)
                nc.tensor.matmul(S_i[:], identb[:], XIB[:, b, :], start=st, stop=sp)
                nc.tensor.matmul(S_rr[:], identb[:], sq_r[:, q, :], start=st, stop=sp)
                nc.tensor.matmul(S_ii[:], identb[:], sq_i[:, q, :], start=st, stop=sp)
                nc.tensor.matmul(S_ri[:], identb[:], p_g[:, q, :], start=st, stop=sp)

        # --- coefficient phase (per-position math on [P, F] f32 tiles) ---
        cp = lambda tag: cpool.tile([P, F], f32, tag=tag, name=f"{tag}{t}")
        scr = lambda i: cpool.tile([P, F], f32, tag=f"scr{i}", name=f"scr{i}_{t}")

        mu_r = cp("mu_r")
        nc.scalar.activation(mu_r[:], S_r[:], ACT.Copy, scale=inv16)
        mu_i = cp("mu_i")
        nc.scalar.activation(mu_i[:], S_i[:], ACT.Copy, scale=inv16)
        Vrr = cp("Vrr")
        nc.scalar.activation(Vrr[:], S_rr[:], ACT.Copy, bias=EPS, scale=inv16)
        Vii = cp("Vii")
        nc.scalar.activation(Vii[:], S_ii[:], ACT.Copy, bias=EPS, scale=inv16)
        Vri = cp("Vri")
        nc.scalar.activation(Vri[:], S_ri[:], ACT.Copy, scale=inv16)

        mr2 = scr(0)
        nc.scalar.square(mr2[:], mu_r[:])
        nc.vector.tensor_sub(Vrr[:], Vrr[:], mr2[:])
        mi2 = scr(1)
        nc.scalar.square(mi2[:], mu_i[:])
        nc.vector.tensor_sub(Vii[:], Vii[:], mi2[:])
        mri = scr(2)
        nc.vector.tensor_mul(mri[:], mu_r[:], mu_i[:])
        nc.vector.tensor_sub(Vri[:], Vri[:], mri[:])

        tau = scr(3)
        nc.vector.tensor_add(tau[:], Vrr[:], Vii[:])
        det = scr(4)
        nc.vector.tensor_mul(det[:], Vrr[:], Vii[:])
        vri2 = scr(1)
        nc.scalar.square(vri2[:], Vri[:])
        nc.vector.tensor_sub(det[:], det[:], vri2[:])

        s_s = cp("s_s")
        nc.scalar.sqrt(s_s[:], det[:])
        # tau <- tau + 2*s in one fused op
        nc.vector.scalar_tensor_tensor(
            tau[:], s_s[:], 2.0, tau[:], mybir.AluOpType.mult, mybir.AluOpType.add
        )
        t_t = cp("t_t")
        nc.scalar.sqrt(t_t[:], tau[:])

        st_ = scr(0)
        nc.vector.tensor_mul(st_[:], s_s[:], t_t[:])
        inv = cp("inv")
        nc.vector.reciprocal_approx_fast(inv[:], st_[:])

        # W matrix in place: Wrr <- Vii, Wii <- Vrr, Wri <- Vri
        # (Wri holds +Vri*inv; true Wri = -that)
        nc.vector.tensor_add(Vii[:], Vii[:], s_s[:])
        nc.vector.tensor_mul(Vii[:], Vii[:], inv[:])
        Wrr = Vii
        nc.vector.tensor_add(Vrr[:], Vrr[:], s_s[:])
        nc.vector.tensor_mul(Vrr[:], Vrr[:], inv[:])
        Wii = Vrr
        nc.vector.tensor_mul(Vri[:], Vri[:], inv[:])
        Wri = Vri

        # output coefficients
        a1 = cp("a1")
        nc.vector.tensor_mul(a1[:], grr[:], Wrr[:])
        m2 = cp("m2")
        nc.vector.tensor_mul(m2[:], gri[:], Wri[:])
        nc.vector.tensor_sub(a1[:], a1[:], m2[:])

        a1b = cbpool.tile([P, F], bf16, tag="a1b", name=f"a1b{t}")
        nc.scalar.copy(a1b[:], a1[:])

        a2 = cp("a2")
        nc.vector.tensor_mul(a2[:], gri[:], Wii[:])
        m4 = scr(1)
        nc.vector.tensor_mul(m4[:], grr[:], Wri[:])
        nc.vector.tensor_sub(a2[:], a2[:], m4[:])

        a2b = cbpool.tile([P, F], bf16, tag="a2b", name=f"a2b{t}")
        nc.scalar.copy(a2b[:], a2[:])

        b1 = cp("b1")
        nc.vector.tensor_mul(b1[:], gri[:], Wrr[:])
        m6 = scr(2)
        nc.vector.tensor_mul(m6[:], gii[:], Wri[:])
        nc.vector.tensor_sub(b1[:], b1[:], m6[:])

        b1b = cbpool.tile([P, F], bf16, tag="b1b", name=f"b1b{t}")
        nc.scalar.copy(b1b[:], b1[:])

        b2 = cp("b2")
        nc.vector.tensor_mul(b2[:], gii[:], Wii[:])
        nc.vector.tensor_sub(b2[:], b2[:], m2[:])

        b2b = cbpool.tile([P, F], bf16, tag="b2b", name=f"b2b{t}")
        nc.scalar.copy(b2b[:], b2[:])

        a0 = cpool.tile([P, F], f32, tag="a0", name=f"a0{t}", bufs=2)
        n1 = scr(3)
        nc.vector.tensor_mul(n1[:], a1[:], mu_r[:])
        nc.vector.tensor_sub(a0[:], bet[:], n1[:])
        n2 = scr(4)
        nc.vector.tensor_mul(n2[:], a2[:], mu_i[:])
        nc.vector.tensor_sub(a0[:], a0[:], n2[:])

        a0b = cbpool.tile([P, F], bf16, tag="a0b", name=f"a0b{t}")
        nc.scalar.copy(a0b[:], a0[:])

        b0 = cpool.tile([P, F], f32, tag="b0", name=f"b0{t}", bufs=2)
        n3 = scr(1)
        nc.vector.tensor_mul(n3[:], b1[:], mu_r[:])
        nc.vector.tensor_sub(b0[:], bet[:], n3[:])
        n4 = scr(0)
        nc.vector.tensor_mul(n4[:], b2[:], mu_i[:])
        nc.vector.tensor_sub(b0[:], b0[:], n4[:])


        b0b = cbpool.tile([P, F], bf16, tag="b0b", name=f"b0b{t}")
        nc.scalar.copy(b0b[:], b0[:])

        # --- pass 2: batched bf16, half the batch per instruction ---
        for h in range(2):
            b0_ = h * HB
            XBh = XB[:, b0_ : b0_ + HB, :]
            XIBh = XIB[:, b0_ : b0_ + HB, :]

            U = wpool.tile([P, HB, F], bf16, tag="U", name=f"U{t}_{h}")
            nc.vector.tensor_mul(U[:], XBh, bcast_free(a1b[:], HB))
            V = wpool.tile([P, HB, F], bf16, tag="V", name=f"V{t}_{h}", bufs=2)
            nc.vector.tensor_mul(V[:], XIBh, bcast_free(a2b[:], HB))

            U2 = wpool.tile([P, HB, F], bf16, tag="U", name=f"U2{t}_{h}")
            nc.vector.tensor_mul(U2[:], XBh, bcast_free(b1b[:], HB))
            V2 = wpool.tile([P, HB, F], bf16, tag="V", name=f"V2{t}_{h}", bufs=2)
            nc.vector.tensor_mul(V2[:], XIBh, bcast_free(b2b[:], HB))

            nfin = 0
            for bb in range(HB):
                b = b0_ + bb
                out_c = opool.tile([P, 2 * F], f32, tag="oc", name=f"oc{t}_{b}")
                oc = out_c.rearrange("p (f two) -> p f two", two=2)
                dve_share = 4 if t < NT - 1 else 7
                for comp, (Uc, Vc, cc) in enumerate(((U, V, a0b), (U2, V2, b0b))):
                    # U+V+bias summed on the TensorEngine into PSUM
                    PS = psum.tile(
                        [P, F], f32, tag="PS", name=f"PS{t}_{b}_{comp}", bufs=3
                    )
                    nc.tensor.matmul(
                        PS[:], identb[:], Uc[:, bb, :], start=True, stop=False
                    )
                    nc.tensor.matmul(
                        PS[:], identb[:], Vc[:, bb, :], start=False, stop=False
                    )
                    nc.tensor.matmul(
                        PS[:], identb[:], cc[:], start=False, stop=True
                    )
                    # strided f32 interleave copy from PSUM; split ACT/DVE
                    if nfin % 8 < 8 - dve_share:
                        nc.scalar.copy(oc[:, :, comp], PS[:])
                    else:
                        nc.vector.tensor_copy(oc[:, :, comp], PS[:])
                    nfin += 1
                nc.sync.dma_start(out_v[b, t], out_c[:])


def build_nc(npos: int = NPOS_FULL, finals_engine: str = "gpsimd") -> bacc.Bacc:
    nc = bacc.Bacc("TRN2", target_bir_lowering=False, debug=False)
    with tile.TileContext(nc) as tc:
        with ExitStack() as ctx:
            _emit(nc, ctx, tc, npos, finals_engine=finals_engine)
    nc.compile()
    return nc


_cache: dict = {}


def _get_nc(npos: int, finals_engine: str = "gpsimd") -> bacc.Bacc:
    key = (npos, finals_engine)
    if key not in _cache:
        _cache[key] = build_nc(npos, finals_engine)
    return _cache[key]


def make_in_maps(x_real, x_imag, gamma_rr, gamma_ri, gamma_ii, beta):
    """Shard channels across cores; returns per-core input dicts."""
    in_maps = []
    for k in range(N_CORES):
        sl = slice(k * C_LOC, (k + 1) * C_LOC)
        in_maps.append(
            {
                "xr": np.ascontiguousarray(x_real[:, sl]).reshape(NB, -1),
                "xi": np.ascontiguousarray(x_imag[:, sl]).reshape(NB, -1),
                "grr": np.ascontiguousarray(gamma_rr[sl]).reshape(-1),
                "gri": np.ascontiguousarray(gamma_ri[sl]).reshape(-1),
                "gii": np.ascontiguousarray(gamma_ii[sl]).reshape(-1),
                "bet": np.ascontiguousarray(beta[sl]).reshape(-1),
            }
        )
    return in_maps


def assemble_output(results) -> np.ndarray:
    """Gather per-core interleaved f32 outputs into the full complex64 array."""
    B = NB
    out = np.empty((B, C_FULL, HW), dtype=np.complex64)
    for k in range(N_CORES):
        o = np.asarray(results[k]["out"])  # [B, 2*NPOS] f32
        oc = o.view(np.complex64).reshape(B, C_LOC, HW)
        out[:, k * C_LOC : (k + 1) * C_LOC] = oc
    return out.reshape(B, C_FULL, 256, 256)


def kernel(x_real, x_imag, gamma_rr, gamma_ri, gamma_ii, beta) -> np.ndarray:
    x_real = np.asarray(x_real, dtype=np.float32)
    x_imag = np.asarray(x_imag, dtype=np.float32)
    gamma_rr = np.asarray(gamma_rr, dtype=np.float32)
    gamma_ri = np.asarray(gamma_ri, dtype=np.float32)
    gamma_ii = np.asarray(gamma_ii, dtype=np.float32)
    beta = np.asarray(beta, dtype=np.float32)

    nc = _get_nc(NPOS_FULL)
    in_maps = make_in_maps(x_real, x_imag, gamma_rr, gamma_ri, gamma_ii, beta)
    res = run_bass_kernel_spmd(nc, in_maps, core_ids=list(range(N_CORES)))
    return assemble_output(res.results)


# revision 23
# speedup vs baseline: 1.1427x; 1.0242x over previous
"""Complex batch-norm Trainium2 kernel (nn_ComplexBatchNormal).

Full inputs: x_real/x_imag [16, 32, 256, 256] f32, params [32, 256, 256] f32.
Output: complex64 [16, 32, 256, 256].

Sharding: channels C=32 split across 8 cores (4 channels each) -> fully local
batch statistics per core, no collectives.

Per-core algorithm (positions N = 4*256*256 = 262144, batch B = 16):
  pass 1: S_r, S_i, S_rr, S_ii, S_ri per position, accumulated over B via
          TensorE identity-matmuls into PSUM (ScalarE squares, VectorE product).
  coef:   analytic inverse-sqrt of the 2x2 covariance; fold gamma/beta/mu into
          6 per-position coefficients a1,a2,a0,b1,b2,b0 with
          out_r = a1*x_r + a2*x_i + a0, out_i = b1*x_r + b2*x_i + b0.
  pass 2: fp16, batched over half the batch per instruction with step-0
          broadcast APs for the coefficients (DVE 2x mode); bias folded into
          the batched chain; strided fp16->f32 cast-copies (split ScalarE/
          VectorE) emit interleaved (re,im) pairs so the DRAM output is
          directly complex64.
"""

import sys

if "/opt/trn_rl_repo" not in sys.path:
    sys.path.insert(0, "/opt/trn_rl_repo")

from contextlib import ExitStack

import numpy as np

import concourse.bacc as bacc
import concourse.bass as bass
import concourse.tile as tile
from concourse import masks, mybir
from concourse.bass_utils import run_bass_kernel_spmd

P = 128          # SBUF partitions
F = 512          # free-dim positions per tile (= one PSUM bank of f32)
NB = 16          # batch size
HB = NB // 2     # half-batch group for batched pass-2 ops
EPS = 1e-5
N_CORES = 8
C_FULL = 32
C_LOC = C_FULL // N_CORES  # 4 channels per core
HW = 256 * 256
NPOS_FULL = C_LOC * HW     # 262144 positions per core

f32 = mybir.dt.float32
bf16 = mybir.dt.float16  # 16-bit compute dtype for pass 2 (fp16: 10-bit mantissa)


def bcast_free(ap: bass.AP, n: int) -> bass.AP:
    """View [P, F] as [P, n, F] with the middle dim broadcast (step 0)."""
    return bass.AP(tensor=ap.tensor, offset=ap.offset, ap=[ap.ap[0], [0, n], ap.ap[1]])


def _emit(
    nc: bacc.Bacc,
    ctx: ExitStack,
    tc: "tile.TileContext",
    npos: int,
    finals_engine: str = "gpsimd",
):
    NT = npos // (P * F)
    assert NT * P * F == npos

    xr_d = nc.dram_tensor("xr", [NB, npos], f32, kind="ExternalInput")
    xi_d = nc.dram_tensor("xi", [NB, npos], f32, kind="ExternalInput")
    grr_d = nc.dram_tensor("grr", [npos], f32, kind="ExternalInput")
    gri_d = nc.dram_tensor("gri", [npos], f32, kind="ExternalInput")
    gii_d = nc.dram_tensor("gii", [npos], f32, kind="ExternalInput")
    bet_d = nc.dram_tensor("bet", [npos], f32, kind="ExternalInput")
    out_d = nc.dram_tensor("out", [NB, 2 * npos], f32, kind="ExternalOutput")

    G = 4  # batch-samples per load group
    xr_gv = xr_d.ap().rearrange("(g q) (t p f) -> g t p q f", q=G, p=P, f=F)
    xi_gv = xi_d.ap().rearrange("(g q) (t p f) -> g t p q f", q=G, p=P, f=F)
    grr_v = grr_d.ap().rearrange("(t p f) -> t p f", p=P, f=F)
    gri_v = gri_d.ap().rearrange("(t p f) -> t p f", p=P, f=F)
    gii_v = gii_d.ap().rearrange("(t p f) -> t p f", p=P, f=F)
    bet_v = bet_d.ap().rearrange("(t p f) -> t p f", p=P, f=F)
    out_v = out_d.ap().rearrange("b (t p f) -> b t p f", p=P, f=2 * F)

    singles = ctx.enter_context(tc.tile_pool(name="singles", bufs=1))
    xpool = ctx.enter_context(tc.tile_pool(name="x", bufs=2))     # f32 staging groups
    xbpool = ctx.enter_context(tc.tile_pool(name="xb", bufs=2))   # fp16 resident
    sqpool = ctx.enter_context(tc.tile_pool(name="sq", bufs=1))
    gpool = ctx.enter_context(tc.tile_pool(name="g", bufs=1))
    cpool = ctx.enter_context(tc.tile_pool(name="coef", bufs=1))
    cbpool = ctx.enter_context(tc.tile_pool(name="coefb", bufs=1))
    wpool = ctx.enter_context(tc.tile_pool(name="w", bufs=2))
    opool = ctx.enter_context(tc.tile_pool(name="o", bufs=2))
    psum = ctx.enter_context(tc.tile_pool(name="ps", bufs=1, space="PSUM"))

    ident = singles.tile([P, P], f32)
    masks.make_identity(nc, ident[:])
    identb = singles.tile([P, P], bf16)
    nc.scalar.copy(identb[:], ident[:])

    inv16 = 1.0 / NB
    ACT = mybir.ActivationFunctionType

    for t in range(NT):
        # --- params for this position tile ---
        grr = gpool.tile([P, F], f32, tag="grr", name=f"grr{t}")
        gri = gpool.tile([P, F], f32, tag="gri", name=f"gri{t}")
        gii = gpool.tile([P, F], f32, tag="gii", name=f"gii{t}")
        bet = gpool.tile([P, F], f32, tag="bet", name=f"bet{t}")
        nc.sync.dma_start(grr[:], grr_v[t])
        nc.sync.dma_start(gri[:], gri_v[t])
        nc.sync.dma_start(gii[:], gii_v[t])
        nc.sync.dma_start(bet[:], bet_v[t])

        # --- pass 1: load x, cast to bf16, accumulate 5 stats over B in PSUM ---
        S_r = psum.tile([P, F], f32, tag="S_r", name=f"S_r{t}")
        S_i = psum.tile([P, F], f32, tag="S_i", name=f"S_i{t}")
        S_rr = psum.tile([P, F], f32, tag="S_rr", name=f"S_rr{t}")
        S_ii = psum.tile([P, F], f32, tag="S_ii", name=f"S_ii{t}")
        S_ri = psum.tile([P, F], f32, tag="S_ri", name=f"S_ri{t}")

        XB = xbpool.tile([P, NB, F], bf16, tag="XB", name=f"XB{t}")
        XIB = xbpool.tile([P, NB, F], bf16, tag="XIB", name=f"XIB{t}")

        for g in range(NB // G):
            xg = xpool.tile([P, G, F], f32, tag="xr", name=f"xr{t}_{g}")
            nc.sync.dma_start(xg[:], xr_gv[g, t])
            yg = xpool.tile([P, G, F], f32, tag="xi", name=f"xi{t}_{g}")
            nc.sync.dma_start(yg[:], xi_gv[g, t])

            XBg = XB[:, g * G : (g + 1) * G, :]
            XIBg = XIB[:, g * G : (g + 1) * G, :]
            nc.scalar.copy(XBg, xg[:])
            nc.scalar.copy(XIBg, yg[:])

            sq_r = sqpool.tile([P, G, F], bf16, tag="sqr", name=f"sqr{t}_{g}")
            sq_i = sqpool.tile([P, G, F], bf16, tag="sqi", name=f"sqi{t}_{g}")
            if t == 0:
                # DVE is idle during the first tile's stats: square there
                nc.vector.tensor_mul(sq_r[:], XBg, XBg)
                nc.vector.tensor_mul(sq_i[:], XIBg, XIBg)
            else:
                nc.scalar.square(sq_r[:], xg[:])
                nc.scalar.square(sq_i[:], yg[:])
            p_g = sqpool.tile([P, G, F], bf16, tag="pg", name=f"pg{t}_{g}")
            nc.vector.tensor_mul(p_g[:], XBg, XIBg)

            for q in range(G):
                b = g * G + q
                st = b == 0
                sp = b == NB - 1
                nc.tensor.matmul(S_r[:], identb[:], XB[:, b, :], start=st, stop=sp)
                nc.tensor.matmul(S_i[:], identb[:], XIB[:, b, :], start=st, stop=sp)
                nc.tensor.matmul(S_rr[:], identb[:], sq_r[:, q, :], start=st, stop=sp)
                nc.tensor.matmul(S_ii[:], identb[:], sq_i[:, q, :], start=st, stop=sp)
                nc.tensor.matmul(S_ri[:], identb[:], p_g[:, q, :], start=st, stop=sp)

        # --- coefficient phase (per-position math on [P, F] f32 tiles) ---
        cp = lambda tag: cpool.tile([P, F], f32, tag=tag, name=f"{tag}{t}")
        scr = lambda i: cpool.tile([P, F], f32, tag=f"scr{i}", name=f"scr{i}_{t}")

        mu_r = cp("mu_r")
        nc.scalar.activation(mu_r[:], S_r[:], ACT.Copy, scale=inv16)
        mu_i = cp("mu_i")
        nc.scalar.activation(mu_i[:], S_i[:], ACT.Copy, scale=inv16)
        Vrr = cp("Vrr")
        nc.scalar.activation(Vrr[:], S_rr[:], ACT.Copy, bias=EPS, scale=inv16)
        Vii = cp("Vii")
        nc.scalar.activation(Vii[:], S_ii[:], ACT.Copy, bias=EPS, scale=inv16)
        Vri = cp("Vri")
        nc.scalar.activation(Vri[:], S_ri[:], ACT.Copy, scale=inv16)

        mr2 = scr(0)
        nc.scalar.square(mr2[:], mu_r[:])
        nc.vector.tensor_sub(Vrr[:], Vrr[:], mr2[:])
        mi2 = scr(1)
        nc.scalar.square(mi2[:], mu_i[:])
        nc.vector.tensor_sub(Vii[:], Vii[:], mi2[:])
        mri = scr(2)
        nc.vector.tensor_mul(mri[:], mu_r[:], mu_i[:])
        nc.vector.tensor_sub(Vri[:], Vri[:], mri[:])

        tau = scr(3)
        nc.vector.tensor_add(tau[:], Vrr[:], Vii[:])
        det = scr(4)
        nc.vector.tensor_mul(det[:], Vrr[:], Vii[:])
        vri2 = scr(1)
        nc.scalar.square(vri2[:], Vri[:])
        nc.vector.tensor_sub(det[:], det[:], vri2[:])

        s_s = cp("s_s")
        nc.scalar.sqrt(s_s[:], det[:])
        # tau <- tau + 2*s in one fused op
        nc.vector.scalar_tensor_tensor(
            tau[:], s_s[:], 2.0, tau[:], mybir.AluOpType.mult, mybir.AluOpType.add
        )
        t_t = cp("t_t")
        nc.scalar.sqrt(t_t[:], tau[:])

        st_ = scr(0)
        nc.vector.tensor_mul(st_[:], s_s[:], t_t[:])
        inv = cp("inv")
        nc.vector.reciprocal_approx_fast(inv[:], st_[:])

        # W matrix in place: Wrr <- Vii, Wii <- Vrr, Wri <- Vri
        # (Wri holds +Vri*inv; true Wri = -that)
        nc.vector.tensor_add(Vii[:], Vii[:], s_s[:])
        nc.vector.tensor_mul(Vii[:], Vii[:], inv[:])
        Wrr = Vii
        nc.vector.tensor_add(Vrr[:], Vrr[:], s_s[:])
        nc.vector.tensor_mul(Vrr[:], Vrr[:], inv[:])
        Wii = Vrr
        nc.vector.tensor_mul(Vri[:], Vri[:], inv[:])
        Wri = Vri

        # output coefficients
        a1 = cp("a1")
        nc.vector.tensor_mul(a1[:], grr[:], Wrr[:])
        m2 = cp("m2")
        nc.vector.tensor_mul(m2[:], gri[:], Wri[:])
        nc.vector.tensor_sub(a1[:], a1[:], m2[:])

        a1b = cbpool.tile([P, F], bf16, tag="a1b", name=f"a1b{t}")
        nc.scalar.copy(a1b[:], a1[:])

        a2 = cp("a2")
        nc.vector.tensor_mul(a2[:], gri[:], Wii[:])
        m4 = scr(1)
        nc.vector.tensor_mul(m4[:], grr[:], Wri[:])
        nc.vector.tensor_sub(a2[:], a2[:], m4[:])

        a2b = cbpool.tile([P, F], bf16, tag="a2b", name=f"a2b{t}")
        nc.scalar.copy(a2b[:], a2[:])

        b1 = cp("b1")
        nc.vector.tensor_mul(b1[:], gri[:], Wrr[:])
        m6 = scr(2)
        nc.vector.tensor_mul(m6[:], gii[:], Wri[:])
        nc.vector.tensor_sub(b1[:], b1[:], m6[:])

        b1b = cbpool.tile([P, F], bf16, tag="b1b", name=f"b1b{t}")
        nc.scalar.copy(b1b[:], b1[:])

        b2 = cp("b2")
        nc.vector.tensor_mul(b2[:], gii[:], Wii[:])
        nc.vector.tensor_sub(b2[:], b2[:], m2[:])

        b2b = cbpool.tile([P, F], bf16, tag="b2b", name=f"b2b{t}")
        nc.scalar.copy(b2b[:], b2[:])

        a0 = cpool.tile([P, F], f32, tag="a0", name=f"a0{t}", bufs=2)
        n1 = scr(3)
        nc.vector.tensor_mul(n1[:], a1[:], mu_r[:])
        nc.vector.tensor_sub(a0[:], bet[:], n1[:])
        n2 = scr(4)
        nc.vector.tensor_mul(n2[:], a2[:], mu_i[:])
        nc.vector.tensor_sub(a0[:], a0[:], n2[:])

        a0b = cbpool.tile([P, F], bf16, tag="a0b", name=f"a0b{t}")
        nc.scalar.copy(a0b[:], a0[:])

        b0 = cpool.tile([P, F], f32, tag="b0", name=f"b0{t}", bufs=2)
        n3 = scr(1)
        nc.vector.tensor_mul(n3[:], b1[:], mu_r[:])
        nc.vector.tensor_sub(b0[:], bet[:], n3[:])
        n4 = scr(0)
        nc.vector.tensor_mul(n4[:], b2[:], mu_i[:])
        nc.vector.tensor_sub(b0[:], b0[:], n4[:])


        b0b = cbpool.tile([P, F], bf16, tag="b0b", name=f"b0b{t}")
        nc.scalar.copy(b0b[:], b0[:])

        # --- pass 2: batched bf16, half the batch per instruction ---
        for h in range(2):
            b0_ = h * HB
            XBh = XB[:, b0_ : b0_ + HB, :]
            XIBh = XIB[:, b0_ : b0_ + HB, :]

            U = wpool.tile([P, HB, F], bf16, tag="U", name=f"U{t}_{h}")
            nc.vector.tensor_mul(U[:], XBh, bcast_free(a1b[:], HB))
            V = wpool.tile([P, HB, F], bf16, tag="V", name=f"V{t}_{h}", bufs=2)
            nc.vector.tensor_mul(V[:], XIBh, bcast_free(a2b[:], HB))

            U2 = wpool.tile([P, HB, F], bf16, tag="U", name=f"U2{t}_{h}")
            nc.vector.tensor_mul(U2[:], XBh, bcast_free(b1b[:], HB))
            V2 = wpool.tile([P, HB, F], bf16, tag="V", name=f"V2{t}_{h}", bufs=2)
            nc.vector.tensor_mul(V2[:], XIBh, bcast_free(b2b[:], HB))

            nfin = 0
            for bb in range(HB):
                b = b0_ + bb
                out_c = opool.tile([P, 2 * F], f32, tag="oc", name=f"oc{t}_{b}")
                oc = out_c.rearrange("p (f two) -> p f two", two=2)
                dve_share = 3 if t < NT - 1 else 7
                for comp, (Uc, Vc, cc) in enumerate(((U, V, a0b), (U2, V2, b0b))):
                    # U+V+bias summed on the TensorEngine into PSUM
                    PS = psum.tile(
                        [P, F], f32, tag="PS", name=f"PS{t}_{b}_{comp}", bufs=3
                    )
                    nc.tensor.matmul(
                        PS[:], identb[:], Uc[:, bb, :], start=True, stop=False
                    )
                    nc.tensor.matmul(
                        PS[:], identb[:], Vc[:, bb, :], start=False, stop=False
                    )
                    nc.tensor.matmul(
                        PS[:], identb[:], cc[:], start=False, stop=True
                    )
                    # strided f32 interleave copy from PSUM; split ACT/DVE
                    if nfin % 8 < 8 - dve_share:
                        nc.scalar.copy(oc[:, :, comp], PS[:])
                    else:
                        nc.vector.tensor_copy(oc[:, :, comp], PS[:])
                    nfin += 1
                nc.sync.dma_start(out_v[b, t], out_c[:])


def build_nc(npos: int = NPOS_FULL, finals_engine: str = "gpsimd") -> bacc.Bacc:
    nc = bacc.Bacc("TRN2", target_bir_lowering=False, debug=False)
    with tile.TileContext(nc) as tc:
        with ExitStack() as ctx:
            _emit(nc, ctx, tc, npos, finals_engine=finals_engine)
    nc.compile()
    return nc


_cache: dict = {}


def _get_nc(npos: int, finals_engine: str = "gpsimd") -> bacc.Bacc:
    key = (npos, finals_engine)
    if key not in _cache:
        _cache[key] = build_nc(npos, finals_engine)
    return _cache[key]


def make_in_maps(x_real, x_imag, gamma_rr, gamma_ri, gamma_ii, beta):
    """Shard channels across cores; returns per-core input dicts."""
    in_maps = []
    for k in range(N_CORES):
        sl = slice(k * C_LOC, (k + 1) * C_LOC)
        in_maps.append(
            {
                "xr": np.ascontiguousarray(x_real[:, sl]).reshape(NB, -1),
                "xi": np.ascontiguousarray(x_imag[:, sl]).reshape(NB, -1),
                "grr": np.ascontiguousarray(gamma_rr[sl]).reshape(-1),
                "gri": np.ascontiguousarray(gamma_ri[sl]).reshape(-1),
                "gii": np.ascontiguousarray(gamma_ii[sl]).reshape(-1),
                "bet": np.ascontiguousarray(beta[sl]).reshape(-1),
            }
        )
    return in_maps


def assemble_output(results) -> np.ndarray:
    """Gather per-core interleaved f32 outputs into the full complex64 array."""
    B = NB
    out = np.empty((B, C_FULL, HW), dtype=np.complex64)
    for k in range(N_CORES):
        o = np.asarray(results[k]["out"])  # [B, 2*NPOS] f32
        oc = o.view(np.complex64).reshape(B, C_LOC, HW)
        out[:, k * C_LOC : (k + 1) * C_LOC] = oc
    return out.reshape(B, C_FULL, 256, 256)


def kernel(x_real, x_imag, gamma_rr, gamma_ri, gamma_ii, beta) -> np.ndarray:
    x_real = np.asarray(x_real, dtype=np.float32)
    x_imag = np.asarray(x_imag, dtype=np.float32)
    gamma_rr = np.asarray(gamma_rr, dtype=np.float32)
    gamma_ri = np.asarray(gamma_ri, dtype=np.float32)
    gamma_ii = np.asarray(gamma_ii, dtype=np.float32)
    beta = np.asarray(beta, dtype=np.float32)

    nc = _get_nc(NPOS_FULL)
    in_maps = make_in_maps(x_real, x_imag, gamma_rr, gamma_ri, gamma_ii, beta)
    res = run_bass_kernel_spmd(nc, in_maps, core_ids=list(range(N_CORES)))
    return assemble_output(res.results)


# revision 24
# speedup vs baseline: 1.1900x; 1.0414x over previous
"""Complex batch-norm Trainium2 kernel (nn_ComplexBatchNormal).

Full inputs: x_real/x_imag [16, 32, 256, 256] f32, params [32, 256, 256] f32.
Output: complex64 [16, 32, 256, 256].

Sharding: channels C=32 split across 8 cores (4 channels each) -> fully local
batch statistics per core, no collectives.

Per-core algorithm (positions N = 4*256*256 = 262144, batch B = 16):
  pass 1: S_r, S_i, S_rr, S_ii, S_ri per position, accumulated over B via
          TensorE identity-matmuls into PSUM (ScalarE squares, VectorE product).
  coef:   analytic inverse-sqrt of the 2x2 covariance; fold gamma/beta/mu into
          6 per-position coefficients a1,a2,a0,b1,b2,b0 with
          out_r = a1*x_r + a2*x_i + a0, out_i = b1*x_r + b2*x_i + b0.
  pass 2: fp16, batched over half the batch per instruction with step-0
          broadcast APs for the coefficients (DVE 2x mode); bias folded into
          the batched chain; strided fp16->f32 cast-copies (split ScalarE/
          VectorE) emit interleaved (re,im) pairs so the DRAM output is
          directly complex64.
"""

import sys

if "/opt/trn_rl_repo" not in sys.path:
    sys.path.insert(0, "/opt/trn_rl_repo")

from contextlib import ExitStack

import numpy as np

import concourse.bacc as bacc
import concourse.bass as bass
import concourse.tile as tile
from concourse import masks, mybir
from concourse.bass_utils import run_bass_kernel_spmd

P = 128          # SBUF partitions
F = 512          # free-dim positions per tile (= one PSUM bank of f32)
NB = 16          # batch size
HB = NB // 2     # half-batch group for batched pass-2 ops
EPS = 1e-5
N_CORES = 8
C_FULL = 32
C_LOC = C_FULL // N_CORES  # 4 channels per core
HW = 256 * 256
NPOS_FULL = C_LOC * HW     # 262144 positions per core

f32 = mybir.dt.float32
bf16 = mybir.dt.float16  # 16-bit compute dtype for pass 2 (fp16: 10-bit mantissa)


def bcast_free(ap: bass.AP, n: int) -> bass.AP:
    """View [P, F] as [P, n, F] with the middle dim broadcast (step 0)."""
    return bass.AP(tensor=ap.tensor, offset=ap.offset, ap=[ap.ap[0], [0, n], ap.ap[1]])


def _emit(
    nc: bacc.Bacc,
    ctx: ExitStack,
    tc: "tile.TileContext",
    npos: int,
    finals_engine: str = "gpsimd",
):
    NT = npos // (P * F)
    assert NT * P * F == npos

    xr_d = nc.dram_tensor("xr", [NB, npos], f32, kind="ExternalInput")
    xi_d = nc.dram_tensor("xi", [NB, npos], f32, kind="ExternalInput")
    grr_d = nc.dram_tensor("grr", [npos], f32, kind="ExternalInput")
    gri_d = nc.dram_tensor("gri", [npos], f32, kind="ExternalInput")
    gii_d = nc.dram_tensor("gii", [npos], f32, kind="ExternalInput")
    bet_d = nc.dram_tensor("bet", [npos], f32, kind="ExternalInput")
    out_d = nc.dram_tensor("out", [NB, 2 * npos], f32, kind="ExternalOutput")

    G = 4  # batch-samples per load group
    xr_gv = xr_d.ap().rearrange("(g q) (t p f) -> g t p q f", q=G, p=P, f=F)
    xi_gv = xi_d.ap().rearrange("(g q) (t p f) -> g t p q f", q=G, p=P, f=F)
    grr_v = grr_d.ap().rearrange("(t p f) -> t p f", p=P, f=F)
    gri_v = gri_d.ap().rearrange("(t p f) -> t p f", p=P, f=F)
    gii_v = gii_d.ap().rearrange("(t p f) -> t p f", p=P, f=F)
    bet_v = bet_d.ap().rearrange("(t p f) -> t p f", p=P, f=F)
    out_v = out_d.ap().rearrange("b (t p f) -> b t p f", p=P, f=2 * F)

    singles = ctx.enter_context(tc.tile_pool(name="singles", bufs=1))
    xpool = ctx.enter_context(tc.tile_pool(name="x", bufs=2))     # f32 staging groups
    xbpool = ctx.enter_context(tc.tile_pool(name="xb", bufs=2))   # fp16 resident
    sqpool = ctx.enter_context(tc.tile_pool(name="sq", bufs=1))
    gpool = ctx.enter_context(tc.tile_pool(name="g", bufs=1))
    cpool = ctx.enter_context(tc.tile_pool(name="coef", bufs=1))
    cbpool = ctx.enter_context(tc.tile_pool(name="coefb", bufs=1))
    wpool = ctx.enter_context(tc.tile_pool(name="w", bufs=2))
    opool = ctx.enter_context(tc.tile_pool(name="o", bufs=2))
    psum = ctx.enter_context(tc.tile_pool(name="ps", bufs=1, space="PSUM"))

    ident = singles.tile([P, P], f32)
    masks.make_identity(nc, ident[:])
    identb = singles.tile([P, P], bf16)
    nc.scalar.copy(identb[:], ident[:])

    inv16 = 1.0 / NB
    ACT = mybir.ActivationFunctionType

    for t in range(NT):
        # --- params for this position tile ---
        grr = gpool.tile([P, F], f32, tag="grr", name=f"grr{t}")
        gri = gpool.tile([P, F], f32, tag="gri", name=f"gii_{t}_r", bufs=1)
        gii = gpool.tile([P, F], f32, tag="gii", name=f"gii{t}")
        bet = gpool.tile([P, F], f32, tag="bet", name=f"bet{t}")

        # --- pass 1: load x, cast to bf16, accumulate 5 stats over B in PSUM ---
        S_r = psum.tile([P, F], f32, tag="S_r", name=f"S_r{t}")
        S_i = psum.tile([P, F], f32, tag="S_i", name=f"S_i{t}")
        S_rr = psum.tile([P, F], f32, tag="S_rr", name=f"S_rr{t}")
        S_ii = psum.tile([P, F], f32, tag="S_ii", name=f"S_ii{t}")
        S_ri = psum.tile([P, F], f32, tag="S_ri", name=f"S_ri{t}")

        XB = xbpool.tile([P, NB, F], bf16, tag="XB", name=f"XB{t}")
        XIB = xbpool.tile([P, NB, F], bf16, tag="XIB", name=f"XIB{t}")

        for g in range(NB // G):
            xg = xpool.tile([P, G, F], f32, tag="xr", name=f"xr{t}_{g}")
            nc.sync.dma_start(xg[:], xr_gv[g, t])
            yg = xpool.tile([P, G, F], f32, tag="xi", name=f"xi{t}_{g}")
            nc.sync.dma_start(yg[:], xi_gv[g, t])

            XBg = XB[:, g * G : (g + 1) * G, :]
            XIBg = XIB[:, g * G : (g + 1) * G, :]
            nc.scalar.copy(XBg, xg[:])
            nc.scalar.copy(XIBg, yg[:])

            sq_r = sqpool.tile([P, G, F], bf16, tag="sqr", name=f"sqr{t}_{g}")
            sq_i = sqpool.tile([P, G, F], bf16, tag="sqi", name=f"sqi{t}_{g}")
            if t == 0:
                # DVE is idle during the first tile's stats: square there
                nc.vector.tensor_mul(sq_r[:], XBg, XBg)
                nc.vector.tensor_mul(sq_i[:], XIBg, XIBg)
            else:
                nc.scalar.square(sq_r[:], xg[:])
                nc.scalar.square(sq_i[:], yg[:])
            p_g = sqpool.tile([P, G, F], bf16, tag="pg", name=f"pg{t}_{g}")
            nc.vector.tensor_mul(p_g[:], XBg, XIBg)

            for q in range(G):
                b = g * G + q
                st = b == 0
                sp = b == NB - 1
                nc.tensor.matmul(S_r[:], identb[:], XB[:, b, :], start=st, stop=sp)
                nc.tensor.matmul(S_i[:], identb[:], XIB[:, b, :], start=st, stop=sp)
                nc.tensor.matmul(S_rr[:], identb[:], sq_r[:, q, :], start=st, stop=sp)
                nc.tensor.matmul(S_ii[:], identb[:], sq_i[:, q, :], start=st, stop=sp)
                nc.tensor.matmul(S_ri[:], identb[:], p_g[:, q, :], start=st, stop=sp)

        # params arrive after x (not needed until the coefficient phase)
        nc.sync.dma_start(grr[:], grr_v[t])
        nc.sync.dma_start(gri[:], gri_v[t])
        nc.sync.dma_start(gii[:], gii_v[t])
        nc.sync.dma_start(bet[:], bet_v[t])

        # --- coefficient phase (per-position math on [P, F] f32 tiles) ---
        cp = lambda tag: cpool.tile([P, F], f32, tag=tag, name=f"{tag}{t}")
        scr = lambda i: cpool.tile([P, F], f32, tag=f"scr{i}", name=f"scr{i}_{t}")

        mu_r = cp("mu_r")
        nc.scalar.activation(mu_r[:], S_r[:], ACT.Copy, scale=inv16)
        mu_i = cp("mu_i")
        nc.scalar.activation(mu_i[:], S_i[:], ACT.Copy, scale=inv16)
        Vrr = cp("Vrr")
        nc.scalar.activation(Vrr[:], S_rr[:], ACT.Copy, bias=EPS, scale=inv16)
        Vii = cp("Vii")
        nc.scalar.activation(Vii[:], S_ii[:], ACT.Copy, bias=EPS, scale=inv16)
        Vri = cp("Vri")
        nc.scalar.activation(Vri[:], S_ri[:], ACT.Copy, scale=inv16)

        mr2 = scr(0)
        nc.scalar.square(mr2[:], mu_r[:])
        nc.vector.tensor_sub(Vrr[:], Vrr[:], mr2[:])
        mi2 = scr(1)
        nc.scalar.square(mi2[:], mu_i[:])
        nc.vector.tensor_sub(Vii[:], Vii[:], mi2[:])
        mri = scr(2)
        nc.vector.tensor_mul(mri[:], mu_r[:], mu_i[:])
        nc.vector.tensor_sub(Vri[:], Vri[:], mri[:])

        tau = scr(3)
        nc.vector.tensor_add(tau[:], Vrr[:], Vii[:])
        det = scr(4)
        nc.vector.tensor_mul(det[:], Vrr[:], Vii[:])
        vri2 = scr(1)
        nc.scalar.square(vri2[:], Vri[:])
        nc.vector.tensor_sub(det[:], det[:], vri2[:])

        s_s = cp("s_s")
        nc.scalar.sqrt(s_s[:], det[:])
        # tau <- tau + 2*s in one fused op
        nc.vector.scalar_tensor_tensor(
            tau[:], s_s[:], 2.0, tau[:], mybir.AluOpType.mult, mybir.AluOpType.add
        )
        t_t = cp("t_t")
        nc.scalar.sqrt(t_t[:], tau[:])

        st_ = scr(0)
        nc.vector.tensor_mul(st_[:], s_s[:], t_t[:])
        inv = cp("inv")
        nc.vector.reciprocal_approx_fast(inv[:], st_[:])

        # W matrix in place: Wrr <- Vii, Wii <- Vrr, Wri <- Vri
        # (Wri holds +Vri*inv; true Wri = -that)
        nc.vector.tensor_add(Vii[:], Vii[:], s_s[:])
        nc.vector.tensor_mul(Vii[:], Vii[:], inv[:])
        Wrr = Vii
        nc.vector.tensor_add(Vrr[:], Vrr[:], s_s[:])
        nc.vector.tensor_mul(Vrr[:], Vrr[:], inv[:])
        Wii = Vrr
        nc.vector.tensor_mul(Vri[:], Vri[:], inv[:])
        Wri = Vri

        # output coefficients
        a1 = cp("a1")
        nc.vector.tensor_mul(a1[:], grr[:], Wrr[:])
        m2 = cp("m2")
        nc.vector.tensor_mul(m2[:], gri[:], Wri[:])
        nc.vector.tensor_sub(a1[:], a1[:], m2[:])

        a1b = cbpool.tile([P, F], bf16, tag="a1b", name=f"a1b{t}")
        nc.scalar.copy(a1b[:], a1[:])

        a2 = cp("a2")
        nc.vector.tensor_mul(a2[:], gri[:], Wii[:])
        m4 = scr(1)
        nc.vector.tensor_mul(m4[:], grr[:], Wri[:])
        nc.vector.tensor_sub(a2[:], a2[:], m4[:])

        a2b = cbpool.tile([P, F], bf16, tag="a2b", name=f"a2b{t}")
        nc.scalar.copy(a2b[:], a2[:])

        b1 = cp("b1")
        nc.vector.tensor_mul(b1[:], gri[:], Wrr[:])
        m6 = scr(2)
        nc.vector.tensor_mul(m6[:], gii[:], Wri[:])
        nc.vector.tensor_sub(b1[:], b1[:], m6[:])

        b1b = cbpool.tile([P, F], bf16, tag="b1b", name=f"b1b{t}")
        nc.scalar.copy(b1b[:], b1[:])

        b2 = cp("b2")
        nc.vector.tensor_mul(b2[:], gii[:], Wii[:])
        nc.vector.tensor_sub(b2[:], b2[:], m2[:])

        b2b = cbpool.tile([P, F], bf16, tag="b2b", name=f"b2b{t}")
        nc.scalar.copy(b2b[:], b2[:])

        a0 = cpool.tile([P, F], f32, tag="a0", name=f"a0{t}", bufs=2)
        n1 = scr(3)
        nc.vector.tensor_mul(n1[:], a1[:], mu_r[:])
        nc.vector.tensor_sub(a0[:], bet[:], n1[:])
        n2 = scr(4)
        nc.vector.tensor_mul(n2[:], a2[:], mu_i[:])
        nc.vector.tensor_sub(a0[:], a0[:], n2[:])

        a0b = cbpool.tile([P, F], bf16, tag="a0b", name=f"a0b{t}")
        nc.scalar.copy(a0b[:], a0[:])

        b0 = cpool.tile([P, F], f32, tag="b0", name=f"b0{t}", bufs=2)
        n3 = scr(1)
        nc.vector.tensor_mul(n3[:], b1[:], mu_r[:])
        nc.vector.tensor_sub(b0[:], bet[:], n3[:])
        n4 = scr(0)
        nc.vector.tensor_mul(n4[:], b2[:], mu_i[:])
        nc.vector.tensor_sub(b0[:], b0[:], n4[:])


        b0b = cbpool.tile([P, F], bf16, tag="b0b", name=f"b0b{t}")
        nc.scalar.copy(b0b[:], b0[:])

        # --- pass 2: batched bf16, half the batch per instruction ---
        for h in range(2):
            b0_ = h * HB
            XBh = XB[:, b0_ : b0_ + HB, :]
            XIBh = XIB[:, b0_ : b0_ + HB, :]

            U = wpool.tile([P, HB, F], bf16, tag="U", name=f"U{t}_{h}")
            nc.vector.tensor_mul(U[:], XBh, bcast_free(a1b[:], HB))
            V = wpool.tile([P, HB, F], bf16, tag="V", name=f"V{t}_{h}", bufs=2)
            nc.vector.tensor_mul(V[:], XIBh, bcast_free(a2b[:], HB))

            U2 = wpool.tile([P, HB, F], bf16, tag="U", name=f"U2{t}_{h}")
            nc.vector.tensor_mul(U2[:], XBh, bcast_free(b1b[:], HB))
            V2 = wpool.tile([P, HB, F], bf16, tag="V", name=f"V2{t}_{h}", bufs=2)
            nc.vector.tensor_mul(V2[:], XIBh, bcast_free(b2b[:], HB))

            nfin = 0
            for bb in range(HB):
                b = b0_ + bb
                out_c = opool.tile([P, 2 * F], f32, tag="oc", name=f"oc{t}_{b}")
                oc = out_c.rearrange("p (f two) -> p f two", two=2)
                dve_share = 2 if t < NT - 1 else 7
                for comp, (Uc, Vc, cc) in enumerate(((U, V, a0b), (U2, V2, b0b))):
                    # U+V+bias summed on the TensorEngine into PSUM
                    PS = psum.tile(
                        [P, F], f32, tag="PS", name=f"PS{t}_{b}_{comp}", bufs=3
                    )
                    nc.tensor.matmul(
                        PS[:], identb[:], Uc[:, bb, :], start=True, stop=False
                    )
                    nc.tensor.matmul(
                        PS[:], identb[:], Vc[:, bb, :], start=False, stop=False
                    )
                    nc.tensor.matmul(
                        PS[:], identb[:], cc[:], start=False, stop=True
                    )
                    # strided f32 interleave copy from PSUM; split ACT/DVE
                    if nfin % 8 < 8 - dve_share:
                        nc.scalar.copy(oc[:, :, comp], PS[:])
                    else:
                        nc.vector.tensor_copy(oc[:, :, comp], PS[:])
                    nfin += 1
                nc.sync.dma_start(out_v[b, t], out_c[:])


def build_nc(npos: int = NPOS_FULL, finals_engine: str = "gpsimd") -> bacc.Bacc:
    nc = bacc.Bacc("TRN2", target_bir_lowering=False, debug=False)
    with tile.TileContext(nc) as tc:
        with ExitStack() as ctx:
            _emit(nc, ctx, tc, npos, finals_engine=finals_engine)
    nc.compile()
    return nc


_cache: dict = {}


def _get_nc(npos: int, finals_engine: str = "gpsimd") -> bacc.Bacc:
    key = (npos, finals_engine)
    if key not in _cache:
        _cache[key] = build_nc(npos, finals_engine)
    return _cache[key]


def make_in_maps(x_real, x_imag, gamma_rr, gamma_ri, gamma_ii, beta):
    """Shard channels across cores; returns per-core input dicts."""
    in_maps = []
    for k in range(N_CORES):
        sl = slice(k * C_LOC, (k + 1) * C_LOC)
        in_maps.append(
            {
                "xr": np.ascontiguousarray(x_real[:, sl]).reshape(NB, -1),
                "xi": np.ascontiguousarray(x_imag[:, sl]).reshape(NB, -1),
                "grr": np.ascontiguousarray(gamma_rr[sl]).reshape(-1),
                "gri": np.ascontiguousarray(gamma_ri[sl]).reshape(-1),
                "gii": np.ascontiguousarray(gamma_ii[sl]).reshape(-1),
                "bet": np.ascontiguousarray(beta[sl]).reshape(-1),
            }
        )
    return in_maps


def assemble_output(results) -> np.ndarray:
    """Gather per-core interleaved f32 outputs into the full complex64 array."""
    B = NB
    out = np.empty((B, C_FULL, HW), dtype=np.complex64)
    for k in range(N_CORES):
        o = np.asarray(results[k]["out"])  # [B, 2*NPOS] f32
        oc = o.view(np.complex64).reshape(B, C_LOC, HW)
        out[:, k * C_LOC : (k + 1) * C_LOC] = oc
    return out.reshape(B, C_FULL, 256, 256)


def kernel(x_real, x_imag, gamma_rr, gamma_ri, gamma_ii, beta) -> np.ndarray:
    x_real = np.asarray(x_real, dtype=np.float32)
    x_imag = np.asarray(x_imag, dtype=np.float32)
    gamma_rr = np.asarray(gamma_rr, dtype=np.float32)
    gamma_ri = np.asarray(gamma_ri, dtype=np.float32)
    gamma_ii = np.asarray(gamma_ii, dtype=np.float32)
    beta = np.asarray(beta, dtype=np.float32)

    nc = _get_nc(NPOS_FULL)
    in_maps = make_in_maps(x_real, x_imag, gamma_rr, gamma_ri, gamma_ii, beta)
    res = run_bass_kernel_spmd(nc, in_maps, core_ids=list(range(N_CORES)))
    return assemble_output(res.results)
